# revision 5
# baseline (speedup 1.0000x reference)
"""DenseSparseGAT (2-layer SpGAT, N=50000, E=800000) on 8 trn2 NeuronCores.

Strategy (graph/data parallel, per sharding hint):
  - Nodes partitioned into 8 contiguous blocks of 6250; edges assigned to the
    core owning their destination (trg) node, sorted by trg within a core.
  - Per-edge source-node features fetched with SWDGE dma_gather from an
    HBM-resident projected-feature table (full table per core = replicated
    "halo"); destination-side attention terms fetched by local trg gather.
  - segment_sum implemented as one-hot matmul into PSUM per 128-node window
    (numerator and denominator accumulated together).
  - Softmax max-subtraction is skipped: alpha = exp(e)/sum(exp(e)) is
    mathematically invariant to the shift and the values are small enough
    that fp32 exp cannot overflow here.
  - Three SPMD launches: L1 projection (sharded), L2 layer-0 edge phase,
    L3 layer-1 (projection + edge phase + log_softmax). Host only does
    layout (concat/transpose/sort/pad) between launches.
"""

import math
import numpy as np

from concourse import bacc, bass, mybir
from concourse.tile import TileContext
from concourse.bass_utils import run_bass_kernel_spmd

DT = mybir.dt.float32
I16 = mybir.dt.int16
AF = mybir.ActivationFunctionType
ALU = mybir.AluOpType

N_CORES = 8


def _cfg_full():
    N = 50000
    NPC = N // N_CORES            # 6250 nodes per core
    W = (NPC + 127) // 128        # 49 windows
    NP = W * 128                  # 6272 padded nodes per core
    NPF = ((N + 127) // 128) * 128  # 50048 padded total nodes
    return dict(N=N, NPC=NPC, W=W, NP=NP, NPF=NPF, SPLIT=N // 2, F=128, H=8, D=16)


def _wrap_idx(a):
    """[num] ints -> dma_gather idx layout [128, num//16] int16 (replicated)."""
    num = a.shape[0]
    assert num % 16 == 0
    blk = np.ascontiguousarray(a.reshape(num // 16, 16).T.astype(np.int16))
    return np.tile(blk, (8, 1))


# --------------------------------------------------------------------------
# kernel builders
# --------------------------------------------------------------------------

def _build_l1(cfg):
    """Projection: hpS_own[n, :144] = [hp(128) | s(8) | t(8)], tS_own[n, :8] = t."""
    W, NP = cfg["W"], cfg["NP"]
    nc = bacc.Bacc(None, target_bir_lowering=False)
    embT = nc.dram_tensor("embT", [128, NP], DT, kind="ExternalInput")
    w0e = nc.dram_tensor("w0e", [128, 144], DT, kind="ExternalInput")
    hpS = nc.dram_tensor("hpS", [NP, 192], DT, kind="ExternalOutput")
    tS = nc.dram_tensor("tS", [NP, 64], DT, kind="ExternalOutput")

    with TileContext(nc) as tc:
        with (
            tc.tile_pool(name="cs", bufs=1) as cs,
            tc.tile_pool(name="sb", bufs=3) as sb,
            tc.tile_pool(name="ps", bufs=2, space="PSUM") as psp,
        ):
            w0t = cs.tile([128, 144], DT)
            nc.sync.dma_start(w0t[:], w0e[:])
            for c in range(W):
                et = sb.tile([128, 128], DT, tag="et")
                nc.sync.dma_start(et[:], embT[:, c * 128:(c + 1) * 128])
                ps = psp.tile([128, 144], DT)
                nc.tensor.matmul(ps[:], lhsT=et[:], rhs=w0t[:], start=True, stop=True)
                ot = sb.tile([128, 144], DT, tag="ot")
                nc.vector.tensor_copy(ot[:], ps[:])
                nc.sync.dma_start(hpS[c * 128:(c + 1) * 128, 0:144], ot[:])
                nc.sync.dma_start(tS[c * 128:(c + 1) * 128, 0:8], ot[:, 136:144])
    nc.compile()
    return nc


def _edge_phase(nc, tc, cfg, src_lo, src_hi, tdram, idxlo, idxhi, idxt, trgl, iota,
                gcols, tcols, scol0, ncol, post_fn):
    """Shared layer edge phase.

    src_lo/src_hi: DRAM tables gathered by (split) global src index, row =
      gcols f32 (msg features at [0:ncol], s at [scol0:scol0+H]).
    tdram: DRAM table gathered by local trg index, row = tcols f32 (t at
      [TC0:TC0+H]).
    post_fn(w, psA, psB): consume the accumulated PSUM tiles.
    Gathers are grouped G windows per call (single_packet=False streams the
    descriptor ring) to amortize SWDGE fixed cost.
    """
    W, H = cfg["W"], cfg["H"]
    W_RUN = cfg.get("W_RUN", W)
    TL, TH = cfg["TL"], cfg["TH"]
    TW = TL + TH
    tc0 = cfg["TC0"]  # column offset of t in the tdram row
    G = cfg.get("G", 2)

    with (
        tc.tile_pool(name="ecs", bufs=1) as cs,
        tc.tile_pool(name="eg", bufs=2) as pg,
        tc.tile_pool(name="esm", bufs=2) as psm,
        tc.tile_pool(name="epp", bufs=4) as pp,
        tc.tile_pool(name="eps", bufs=2, space="PSUM") as psp,
    ):
        ilo = cs.tile([128, W * TL * 8], I16)
        nc.sync.dma_start(ilo[:], idxlo[:])
        ihi = cs.tile([128, W * TH * 8], I16)
        nc.sync.dma_start(ihi[:], idxhi[:])
        it = cs.tile([128, W * TW * 8], I16)
        nc.sync.dma_start(it[:], idxt[:])
        tg = cs.tile([128, W * TW], DT)
        nc.sync.dma_start(tg[:], trgl[:])
        io = cs.tile([128, 128], DT)
        nc.sync.dma_start(io[:], iota[:])

        for w0 in range(0, W_RUN, G):
            gw = min(G, W_RUN - w0)
            glo = pg.tile([128, G * TL, gcols], DT, tag="glo")
            nc.gpsimd.dma_gather(
                glo[:, 0:gw * TL, :], src_lo[:],
                ilo[:, w0 * TL * 8:(w0 + gw) * TL * 8],
                gw * TL * 128, gw * TL * 128, gcols, single_packet=False)
            ghi = pg.tile([128, G * TH, gcols], DT, tag="ghi")
            nc.gpsimd.dma_gather(
                ghi[:, 0:gw * TH, :], src_hi[:],
                ihi[:, w0 * TH * 8:(w0 + gw) * TH * 8],
                gw * TH * 128, gw * TH * 128, gcols, single_packet=False)
            gt = pg.tile([128, G * TW, tcols], DT, tag="gt")
            nc.gpsimd.dma_gather(
                gt[:, 0:gw * TW, :], tdram[:],
                it[:, w0 * TW * 8:(w0 + gw) * TW * 8],
                gw * TW * 128, gw * TW * 128, tcols, single_packet=False)

            for wr in range(gw):
                w = w0 + wr
                # e = leaky_relu(s_src + t_trg, 0.2); ex = exp(e)
                e = psm.tile([128, TW, H], DT, tag="e")
                nc.vector.tensor_tensor(
                    out=e[:, 0:TL, :],
                    in0=glo[:, wr * TL:(wr + 1) * TL, scol0:scol0 + H],
                    in1=gt[:, wr * TW:wr * TW + TL, tc0:tc0 + H], op=ALU.add)
                nc.vector.tensor_tensor(
                    out=e[:, TL:TW, :],
                    in0=ghi[:, wr * TH:(wr + 1) * TH, scol0:scol0 + H],
                    in1=gt[:, wr * TW + TL:(wr + 1) * TW, tc0:tc0 + H], op=ALU.add)
                el = psm.tile([128, TW, H], DT, tag="el")
                nc.vector.scalar_tensor_tensor(
                    out=el[:], in0=e[:], scalar=0.2, in1=e[:],
                    op0=ALU.mult, op1=ALU.max)
                ex = psm.tile([128, TW, H], DT, tag="ex")
                nc.scalar.activation(ex[:], el[:], AF.Exp)

                # msg = hp_src * ex (per head block)
                msg = psm.tile([128, TW, ncol], DT, tag="msg")
                dsub = ncol // H
                nc.vector.tensor_tensor(
                    out=msg[:, 0:TL, :].rearrange("p t (h d) -> p t h d", d=dsub),
                    in0=glo[:, wr * TL:(wr + 1) * TL, 0:ncol]
                        .rearrange("p t (h d) -> p t h d", d=dsub),
                    in1=ex[:, 0:TL, :, None].to_broadcast([128, TL, H, dsub]),
                    op=ALU.mult)
                nc.vector.tensor_tensor(
                    out=msg[:, TL:TW, :].rearrange("p t (h d) -> p t h d", d=dsub),
                    in0=ghi[:, wr * TH:(wr + 1) * TH, 0:ncol]
                        .rearrange("p t (h d) -> p t h d", d=dsub),
                    in1=ex[:, TL:TW, :, None].to_broadcast([128, TH, H, dsub]),
                    op=ALU.mult)

                # segment sum via one-hot matmuls into PSUM (num and den in
                # separate banks so the two accumulation groups don't collide)
                psA = psp.tile([128, ncol], DT, tag="psA")
                psB = psp.tile([128, H], DT, tag="psB")
                for t in range(TW):
                    P = pp.tile([128, 128], DT, tag="P")
                    nc.vector.tensor_scalar(
                        out=P[:], in0=io[:],
                        scalar1=tg[:, w * TW + t:w * TW + t + 1],
                        scalar2=None, op0=ALU.is_equal)
                    nc.tensor.matmul(psA[:], lhsT=P[:], rhs=msg[:, t, :],
                                     start=(t == 0), stop=(t == TW - 1))
                    nc.tensor.matmul(psB[:], lhsT=P[:], rhs=ex[:, t, :],
                                     start=(t == 0), stop=(t == TW - 1))

                post_fn(w, psA, psB)


def _build_l2(cfg):
    """Layer-0 edge phase: gathers from hpS (lo/hi split), t from tS, out h1."""
    W, NP, H = cfg["W"], cfg["NP"], cfg["H"]
    SPLIT, N = cfg["SPLIT"], cfg["N"]
    TL, TH = cfg["TL"], cfg["TH"]
    TW = TL + TH
    nc = bacc.Bacc(None, target_bir_lowering=False)
    hpL = nc.dram_tensor("hpL", [SPLIT, 192], DT, kind="ExternalInput")
    hpH = nc.dram_tensor("hpH", [N - SPLIT, 192], DT, kind="ExternalInput")
    tS = nc.dram_tensor("tS", [NP, 64], DT, kind="ExternalInput")
    idxlo = nc.dram_tensor("idxlo", [128, W * TL * 8], I16, kind="ExternalInput")
    idxhi = nc.dram_tensor("idxhi", [128, W * TH * 8], I16, kind="ExternalInput")
    idxt = nc.dram_tensor("idxt", [128, W * TW * 8], I16, kind="ExternalInput")
    trgl = nc.dram_tensor("trgl", [128, W * TW], DT, kind="ExternalInput")
    iota = nc.dram_tensor("iota", [128, 128], DT, kind="ExternalInput")
    h1 = nc.dram_tensor("h1", [NP, 128], DT, kind="ExternalOutput")

    with TileContext(nc) as tc:
        with tc.tile_pool(name="post", bufs=3) as pq:
            def post(w, psA, psB):
                # h0 = num / (den + 1e-16); h1 = elu(h0)
                dn = pq.tile([128, H], DT, tag="dn")
                nc.vector.tensor_scalar(out=dn[:], in0=psB[:],
                                        scalar1=1e-16, scalar2=None, op0=ALU.add)
                rc = pq.tile([128, H], DT, tag="rc")
                nc.vector.reciprocal(rc[:], dn[:])
                h0 = pq.tile([128, 128], DT, tag="h0")
                nc.vector.tensor_tensor(
                    out=h0[:].rearrange("p (h d) -> p h d", d=16),
                    in0=psA[:].rearrange("p (h d) -> p h d", d=16),
                    in1=rc[:, :, None].to_broadcast([128, H, 16]),
                    op=ALU.mult)
                hm = pq.tile([128, 128], DT, tag="hm")
                nc.vector.tensor_scalar(out=hm[:], in0=h0[:], scalar1=0.0,
                                        scalar2=None, op0=ALU.min)
                he = pq.tile([128, 128], DT, tag="he")
                nc.scalar.activation(he[:], hm[:], AF.Exp)
                ho = pq.tile([128, 128], DT, tag="ho")
                nc.vector.scalar_tensor_tensor(
                    out=ho[:], in0=h0[:], scalar=0.0, in1=he[:],
                    op0=ALU.max, op1=ALU.add)
                nc.vector.tensor_scalar(out=ho[:], in0=ho[:], scalar1=1.0,
                                        scalar2=None, op0=ALU.subtract)
                nc.sync.dma_start(h1[w * 128:(w + 1) * 128, :], ho[:])

            _edge_phase(nc, tc, cfg, hpL, hpH, tS, idxlo, idxhi, idxt, trgl, iota,
                        gcols=192, tcols=64, scol0=128, ncol=128, post_fn=post)
    nc.compile()
    return nc


def _build_l3(cfg):
    """Layer 1: full projection of h1 -> hp1S, edge phase, log_softmax."""
    W, NP, NPF = cfg["W"], cfg["NP"], cfg["NPF"]
    SPLIT = cfg["SPLIT"]
    TL, TH = cfg["TL"], cfg["TH"]
    TW = TL + TH
    CH = NPF // 128
    nc = bacc.Bacc(None, target_bir_lowering=False)
    h1T = nc.dram_tensor("h1T", [128, NPF], DT, kind="ExternalInput")
    h1To = nc.dram_tensor("h1To", [128, NP], DT, kind="ExternalInput")
    w1e = nc.dram_tensor("w1e", [128, 4], DT, kind="ExternalInput")
    idxlo = nc.dram_tensor("idxlo", [128, W * TL * 8], I16, kind="ExternalInput")
    idxhi = nc.dram_tensor("idxhi", [128, W * TH * 8], I16, kind="ExternalInput")
    idxt = nc.dram_tensor("idxt", [128, W * TW * 8], I16, kind="ExternalInput")
    trgl = nc.dram_tensor("trgl", [128, W * TW], DT, kind="ExternalInput")
    iota = nc.dram_tensor("iota", [128, 128], DT, kind="ExternalInput")
    outp = nc.dram_tensor("outp", [128, W, 2], DT, kind="ExternalOutput")

    hp1L = nc.dram_tensor("hp1L", [SPLIT, 64], DT)
    hp1H = nc.dram_tensor("hp1H", [NPF - SPLIT, 64], DT)
    t1S = nc.dram_tensor("t1S", [NP, 64], DT)

    with TileContext(nc) as tc:
        with (
            tc.tile_pool(name="pcs", bufs=1) as cs,
            tc.tile_pool(name="psb", bufs=4) as sb,
            tc.tile_pool(name="pps", bufs=4, space="PSUM") as psp,
            tc.tile_pool(name="post", bufs=1) as pq,
        ):
            w1t = cs.tile([128, 4], DT)
            nc.sync.dma_start(w1t[:], w1e[:])

            # (a) full projection hp1S = [hp1(2) | s1(1) | t1(1)] rows
            for c in range(CH):
                ht = sb.tile([128, 128], DT, tag="ht")
                nc.sync.dma_start(ht[:], h1T[:, c * 128:(c + 1) * 128])
                ps = psp.tile([128, 4], DT, tag="psa")
                nc.tensor.matmul(ps[:], lhsT=ht[:], rhs=w1t[:], start=True, stop=True)
                ot = sb.tile([128, 4], DT, tag="ota")
                nc.vector.tensor_copy(ot[:], ps[:])
                r0, r1 = c * 128, (c + 1) * 128
                if r1 <= SPLIT:
                    nc.sync.dma_start(hp1L[r0:r1, 0:4], ot[:])
                elif r0 >= SPLIT:
                    nc.sync.dma_start(hp1H[r0 - SPLIT:r1 - SPLIT, 0:4], ot[:])
                else:
                    k = SPLIT - r0
                    nc.sync.dma_start(hp1L[r0:SPLIT, 0:4], ot[0:k, :])
                    nc.sync.dma_start(hp1H[0:r1 - SPLIT, 0:4], ot[k:128, :])

            # (b) t1 for own nodes
            for c in range(W):
                ht = sb.tile([128, 128], DT, tag="ht")
                nc.sync.dma_start(ht[:], h1To[:, c * 128:(c + 1) * 128])
                ps = psp.tile([128, 4], DT, tag="psa")
                nc.tensor.matmul(ps[:], lhsT=ht[:], rhs=w1t[:], start=True, stop=True)
                ot = sb.tile([128, 4], DT, tag="ota")
                nc.vector.tensor_copy(ot[:], ps[:])
                nc.sync.dma_start(t1S[c * 128:(c + 1) * 128, 0:4], ot[:])

            # (c) edge phase; accumulate per-window logits
            acc = pq.tile([128, W, 3], DT)

            def post(w, psA, psB):
                nc.vector.tensor_copy(acc[:, w, 0:2], psA[:])
                nc.vector.tensor_copy(acc[:, w, 2:3], psB[:])

            # layer-1: H=1, msg cols = 2, s at col 2, t at col 3 of its row
            cfg3 = dict(cfg)
            cfg3["H"] = 1
            cfg3["TC0"] = 3
            _edge_phase(nc, tc, cfg3, hp1L, hp1H, t1S, idxlo, idxhi, idxt, trgl,
                        iota, gcols=64, tcols=64, scol0=2, ncol=2, post_fn=post)

            # (d) batched normalize + log_softmax
            dn = pq.tile([128, W], DT)
            nc.vector.tensor_scalar(out=dn[:], in0=acc[:, :, 2], scalar1=1e-16,
                                    scalar2=None, op0=ALU.add)
            rc = pq.tile([128, W], DT)
            nc.vector.reciprocal(rc[:], dn[:])
            lg = pq.tile([128, W, 2], DT)
            nc.vector.tensor_tensor(out=lg[:], in0=acc[:, :, 0:2],
                                    in1=rc[:, :, None].to_broadcast([128, W, 2]),
                                    op=ALU.mult)
            mx = pq.tile([128, W], DT)
            nc.vector.tensor_reduce(out=mx[:], in_=lg[:], axis=mybir.AxisListType.X,
                                    op=ALU.max)
            dd = pq.tile([128, W, 2], DT)
            nc.vector.tensor_tensor(out=dd[:], in0=lg[:],
                                    in1=mx[:, :, None].to_broadcast([128, W, 2]),
                                    op=ALU.subtract)
            e2 = pq.tile([128, W, 2], DT)
            nc.scalar.activation(e2[:], dd[:], AF.Exp)
            se = pq.tile([128, W], DT)
            nc.vector.tensor_reduce(out=se[:], in_=e2[:], axis=mybir.AxisListType.X,
                                    op=ALU.add)
            ls = pq.tile([128, W], DT)
            nc.scalar.activation(ls[:], se[:], AF.Ln)
            ov = pq.tile([128, W, 2], DT)
            nc.vector.tensor_tensor(out=ov[:], in0=dd[:],
                                    in1=ls[:, :, None].to_broadcast([128, W, 2]),
                                    op=ALU.subtract)
            nc.sync.dma_start(outp[:], ov[:])
    nc.compile()
    return nc


# --------------------------------------------------------------------------
# host-side prep (index/layout only)
# --------------------------------------------------------------------------

def _prep_edges(src, trg, cfg):
    """Partition+sort edges by destination; build per-core gather index and
    one-hot-builder arrays. Returns (TL, TH, per_core list of dicts)."""
    N, NPC, W, SPLIT = cfg["N"], cfg["NPC"], cfg["W"], cfg["SPLIT"]
    src = np.asarray(src).astype(np.int64)
    trg = np.asarray(trg).astype(np.int64)

    cores = []
    max_lo = 1
    max_hi = 1
    for c in range(N_CORES):
        m = (trg >= c * NPC) & (trg < (c + 1) * NPC)
        es, et = src[m], trg[m] - c * NPC
        o = np.argsort(et, kind="stable")
        es, et = es[o], et[o]
        bounds = np.searchsorted(et, np.arange(W + 1) * 128)
        wins = []
        for w in range(W):
            ws, wt = es[bounds[w]:bounds[w + 1]], et[bounds[w]:bounds[w + 1]]
            lo = ws < SPLIT
            wins.append((ws[lo], wt[lo], ws[~lo] - SPLIT, wt[~lo]))
            max_lo = max(max_lo, int(lo.sum()))
            max_hi = max(max_hi, int((~lo).sum()))
        cores.append(wins)

    TL = (max_lo + 127) // 128
    TH = (max_hi + 127) // 128
    TW = TL + TH

    per_core = []
    for c in range(N_CORES):
        idxlo = np.zeros((W, TL * 128), np.int64)
        idxhi = np.zeros((W, TH * 128), np.int64)
        idxt = np.zeros((W, TW * 128), np.int64)
        trgl = np.full((W, TW * 128), -1.0, np.float32)
        for w, (slo, tlo, shi, thi) in enumerate(cores[c]):
            nlo, nhi = len(slo), len(shi)
            idxlo[w, :nlo] = slo
            idxt[w, :nlo] = tlo
            trgl[w, :nlo] = tlo - 128 * w
            idxhi[w, :nhi] = shi
            idxt[w, TL * 128:TL * 128 + nhi] = thi
            trgl[w, TL * 128:TL * 128 + nhi] = thi - 128 * w
        per_core.append(dict(
            idxlo=np.concatenate([_wrap_idx(idxlo[w]) for w in range(W)], axis=1),
            idxhi=np.concatenate([_wrap_idx(idxhi[w]) for w in range(W)], axis=1),
            idxt=np.concatenate([_wrap_idx(idxt[w]) for w in range(W)], axis=1),
            trgl=np.ascontiguousarray(
                np.stack([trgl[w].reshape(TW, 128).T for w in range(W)], axis=1)
                .reshape(128, W * TW)),
        ))
    return TL, TH, per_core


_NC_CACHE = {}


def _cached(key, build, cfg):
    if key not in _NC_CACHE:
        _NC_CACHE[key] = build(cfg)
    return _NC_CACHE[key]


def _run(nc, in_maps, **kw):
    return run_bass_kernel_spmd(nc, in_maps, list(range(N_CORES)), **kw)


def kernel(static_emb, dyn0, dyn1, src_indices, trg_indices,
           w0, asrc0, atrg0, w1, asrc1, atrg1, _cfg=None, _runner=None):
    cfg = dict(_cfg_full() if _cfg is None else _cfg)
    N, NPC, W, NP, NPF = cfg["N"], cfg["NPC"], cfg["W"], cfg["NP"], cfg["NPF"]
    SPLIT = cfg["SPLIT"]
    run = _runner if _runner is not None else _run

    f32 = np.float32
    emb = np.concatenate([np.asarray(dyn0, f32), np.asarray(dyn1, f32),
                          np.asarray(static_emb, f32)], axis=1)  # [N, 128]
    embT = np.ascontiguousarray(emb.T)  # [128, N]

    w0 = np.asarray(w0, f32)
    w0e = np.zeros((128, 144), f32)
    w0e[:, :128] = w0.transpose(1, 0, 2).reshape(128, 128)
    w0e[:, 128:136] = np.einsum("hfd,hd->fh", w0, np.asarray(asrc0, f32)[:, :, 0])
    w0e[:, 136:144] = np.einsum("hfd,hd->fh", w0, np.asarray(atrg0, f32)[:, :, 0])

    w1 = np.asarray(w1, f32)
    w1e = np.zeros((128, 4), f32)
    w1e[:, 0:2] = w1[0]
    w1e[:, 2] = w1[0] @ np.asarray(asrc1, f32)[0, :, 0]
    w1e[:, 3] = w1[0] @ np.asarray(atrg1, f32)[0, :, 0]

    TL, TH, eprep = _prep_edges(src_indices, trg_indices, cfg)
    cfg["TL"], cfg["TH"] = TL, TH
    cfg["TC0"] = 0  # t column offset in tS rows (layer 0)
    cfg["H"] = 8

    iota = np.broadcast_to(np.arange(128, dtype=f32), (128, 128)).copy()

    # ---- L1: sharded projection -----------------------------------------
    nc1 = _cached(("l1", cfg["NP"]), _build_l1, cfg)
    in1 = []
    for c in range(N_CORES):
        eo = np.zeros((128, NP), f32)
        eo[:, :NPC] = embT[:, c * NPC:(c + 1) * NPC]
        in1.append(dict(embT=eo, w0e=w0e))
    r1 = run(nc1, in1)

    hpS = np.concatenate([r1.results[c]["hpS"][:NPC] for c in range(N_CORES)])
    hpL = np.ascontiguousarray(hpS[:SPLIT])
    hpH = np.ascontiguousarray(hpS[SPLIT:])

    # ---- L2: layer-0 edge phase -----------------------------------------
    nc2 = _cached(("l2", cfg["NP"], TL, TH), _build_l2, cfg)
    in2 = []
    for c in range(N_CORES):
        in2.append(dict(hpL=hpL, hpH=hpH, tS=r1.results[c]["tS"],
                        idxlo=eprep[c]["idxlo"], idxhi=eprep[c]["idxhi"],
                        idxt=eprep[c]["idxt"], trgl=eprep[c]["trgl"], iota=iota))
    r2 = run(nc2, in2)

    h1 = np.zeros((NPF, 128), f32)
    for c in range(N_CORES):
        h1[c * NPC:(c + 1) * NPC] = r2.results[c]["h1"][:NPC]
    h1T = np.ascontiguousarray(h1.T)  # [128, NPF]

    # ---- L3: layer 1 + log_softmax --------------------------------------
    nc3 = _cached(("l3", cfg["NP"], TL, TH), _build_l3, cfg)
    in3 = []
    for c in range(N_CORES):
        ho = np.zeros((128, NP), f32)
        ho[:, :NPC] = h1T[:, c * NPC:(c + 1) * NPC]
        in3.append(dict(h1T=h1T, h1To=ho, w1e=w1e,
                        idxlo=eprep[c]["idxlo"], idxhi=eprep[c]["idxhi"],
                        idxt=eprep[c]["idxt"], trgl=eprep[c]["trgl"], iota=iota))
    r3 = run(nc3, in3)

    out = np.zeros((N, 2), f32)
    for c in range(N_CORES):
        o = r3.results[c]["outp"]  # [128, W, 2]; node = w*128 + p (local)
        loc = np.transpose(o, (1, 0, 2)).reshape(NP, 2)
        out[c * NPC:(c + 1) * NPC] = loc[:NPC]
    return out


# revision 6
# speedup vs baseline: 2.7309x; 2.7309x over previous
"""DenseSparseGAT (2-layer SpGAT, N=50000, E=800000) on 8 trn2 NeuronCores.

Strategy (graph/data parallel, per the sharding hint):
  - Nodes partitioned into 8 contiguous blocks of 6250; edges assigned to the
    core owning their destination (trg) node, sorted by trg within a core.
  - Per-edge source-node features fetched with SWDGE dma_gather from an
    HBM-resident projected-feature table (full table per core = replicated
    halo). Gathers are issued per 128-node window on rotating SWDGE queues
    (4 queues) so descriptor rings drain in parallel.
  - Destination-side attention terms are NOT gathered: within a window the
    trg values span 128 nodes, so t[trg] is expanded with a one-hot matmul
    (P_T[node, edge] built from a broadcast + is_equal, then PE matmul
    against the window's t rows).
  - segment_sum is a one-hot matmul into PSUM per window: lhsT = P[edge,
    node] one-hot (built with one is_equal per 128-edge tile), rhs =
    [msg | exp] packed bf16.
  - Softmax max-subtraction is skipped: alpha = exp(e)/sum(exp(e)) is
    shift-invariant and the scores are small enough that fp32 exp cannot
    overflow here.
  - Three SPMD launches: L1 projection (sharded), L2 layer-0 edge phase,
    L3 layer-1 (projection + edge phase + log_softmax). Host only does
    layout work (concat/transpose/sort/pad) between launches.
"""

import numpy as np

from concourse import bacc, bass, mybir
from concourse.tile import TileContext
from concourse.bass_utils import run_bass_kernel_spmd

DT = mybir.dt.float32
BF = mybir.dt.bfloat16
I16 = mybir.dt.int16
AF = mybir.ActivationFunctionType
ALU = mybir.AluOpType
NPBF = mybir.dt.np(BF)

N_CORES = 8
NQ = 4  # SWDGE queues


def _cfg_full():
    N = 50000
    NPC = N // N_CORES            # 6250 nodes per core
    W = (NPC + 127) // 128        # 49 windows
    NP = W * 128                  # 6272 padded nodes per core
    NPF = ((N + 127) // 128) * 128  # 50048 padded total nodes
    return dict(N=N, NPC=NPC, W=W, NP=NP, NPF=NPF, SPLIT=N // 2, H=8)


def _wrap_idx(a):
    """[num] ints -> dma_gather idx layout [128, num//16] int16 (replicated)."""
    num = a.shape[0]
    assert num % 16 == 0
    blk = np.ascontiguousarray(a.reshape(num // 16, 16).T.astype(np.int16))
    return np.tile(blk, (8, 1))


# --------------------------------------------------------------------------
# kernel builders
# --------------------------------------------------------------------------

def _build_l1(cfg):
    """Projection: hpS[n, :144] = [hp(128) | s(8) | t(8)], tS[n, :8] = t."""
    W, NP = cfg["W"], cfg["NP"]
    nc = bacc.Bacc(None, target_bir_lowering=False)
    embT = nc.dram_tensor("embT", [128, NP], DT, kind="ExternalInput")
    w0e = nc.dram_tensor("w0e", [128, 144], DT, kind="ExternalInput")
    hpS = nc.dram_tensor("hpS", [NP, 192], DT, kind="ExternalOutput")
    tS = nc.dram_tensor("tS", [NP, 64], DT, kind="ExternalOutput")

    B = 4  # chunks per DMA batch
    with TileContext(nc) as tc:
        with (
            tc.tile_pool(name="cs", bufs=1) as cs,
            tc.tile_pool(name="sb", bufs=3) as sb,
            tc.tile_pool(name="ps", bufs=2, space="PSUM") as psp,
        ):
            w0t = cs.tile([128, 144], DT)
            nc.sync.dma_start(w0t[:], w0e[:])
            for c0 in range(0, W, B):
                nb = min(B, W - c0)
                et = sb.tile([128, B * 128], DT, tag="et")
                nc.sync.dma_start(et[:, 0:nb * 128],
                                  embT[:, c0 * 128:(c0 + nb) * 128])
                ot = sb.tile([128, B, 144], DT, tag="ot")
                for i in range(nb):
                    ps = psp.tile([128, 144], DT)
                    nc.tensor.matmul(ps[:], lhsT=et[:, i * 128:(i + 1) * 128],
                                     rhs=w0t[:], start=True, stop=True)
                    nc.vector.tensor_copy(ot[:, i, :], ps[:])
                r0, r1 = c0 * 128, (c0 + nb) * 128
                nc.sync.dma_start(
                    hpS[r0:r1, 0:144].rearrange("(c p) d -> p c d", p=128),
                    ot[:, 0:nb, :])
                nc.scalar.dma_start(
                    tS[r0:r1, 0:8].rearrange("(c p) d -> p c d", p=128),
                    ot[:, 0:nb, 136:144])
    nc.compile()
    return nc


def _edge_phase(nc, tc, cfg, src_lo, src_hi, tdram, idxlo, idxhi, tg_d, tgr_d,
                iota_d, iotac_d, gcols, scol0, ncol, post_fn):
    """Shared edge phase for one GAT layer.

    src_lo/src_hi: DRAM tables gathered by (split) src index; fp32 rows of
      gcols (msg features at [0:ncol], s at [scol0:scol0+H]).
    tdram: fp32 [NP, 64] table, t at cols [0:H], read per window (no gather).
    """
    W, H = cfg["W"], cfg["H"]
    W_RUN = cfg.get("W_RUN", W)
    TL, TH = cfg["TL"], cfg["TH"]
    TW = TL + TH
    EW = TW * 128  # edge slots per window

    with (
        tc.tile_pool(name="ecs", bufs=1) as cs,
        tc.tile_pool(name="eg", bufs=3) as pg,
        tc.tile_pool(name="esm", bufs=2) as psm,
        tc.tile_pool(name="epp", bufs=4) as pp,
        tc.tile_pool(name="eps", bufs=2, space="PSUM") as psp,
    ):
        ilo = cs.tile([128, W * TL * 8], I16)
        nc.sync.dma_start(ilo[:], idxlo[:])
        ihi = cs.tile([128, W * TH * 8], I16)
        nc.sync.dma_start(ihi[:], idxhi[:])
        tg = cs.tile([128, W * TW], DT)
        nc.sync.dma_start(tg[:], tg_d[:])
        io = cs.tile([128, 128], BF)
        nc.sync.dma_start(io[:], iota_d[:])
        ioc = cs.tile([128, 1], DT)
        nc.sync.dma_start(ioc[:], iotac_d[:])
        ones = cs.tile([1, 128], BF)
        nc.gpsimd.memset(ones[:], 1.0)

        for w in range(W_RUN):
            glo = pg.tile([128, TL, gcols], DT, tag="glo")
            nc.gpsimd.dma_gather(
                glo[:], src_lo[:], ilo[:, w * TL * 8:(w + 1) * TL * 8],
                TL * 128, TL * 128, gcols, single_packet=False,
                queue_num=(2 * w) % NQ)
            ghi = pg.tile([128, TH, gcols], DT, tag="ghi")
            nc.gpsimd.dma_gather(
                ghi[:], src_hi[:], ihi[:, w * TH * 8:(w + 1) * TH * 8],
                TH * 128, TH * 128, gcols, single_packet=False,
                queue_num=(2 * w + 1) % NQ)

            # window t rows + trgl broadcast row
            twf = psm.tile([128, H], DT, tag="twf")
            nc.scalar.dma_start(twf[:], tdram[w * 128:(w + 1) * 128, 0:H])
            twb = psm.tile([128, H], BF, tag="twb")
            nc.vector.tensor_copy(twb[:], twf[:])
            tgr = psm.tile([1, EW], BF, tag="tgr")
            nc.scalar.dma_start(tgr[:], tgr_d[:, w * EW:(w + 1) * EW])

            # P_T[node, edge] one-hot: broadcast trgl row, compare to iota col
            PT = psm.tile([128, EW], BF, tag="PT")
            for o in range(0, EW, 512):
                S = min(512, EW - o)
                psb = psp.tile([128, 512], DT, tag="psb")
                nc.tensor.matmul(psb[:, 0:S], lhsT=ones[:], rhs=tgr[:, o:o + S],
                                 start=True, stop=True)
                nc.vector.tensor_scalar(
                    out=PT[:, o:o + S], in0=psb[:, 0:S], scalar1=ioc[:],
                    scalar2=None, op0=ALU.is_equal)

            # t_edge[edge, h] = P_T^T @ t_win, per 128-edge tile into one bank
            psT = psp.tile([128, TW * H], DT, tag="psT")
            for t in range(TW):
                nc.tensor.matmul(psT[:, t * H:(t + 1) * H],
                                 lhsT=PT[:, t * 128:(t + 1) * 128],
                                 rhs=twb[:], start=True, stop=True)

            # e = leaky_relu(s_src + t_trg, 0.2); rhs = [msg | exp] bf16
            e = psm.tile([128, TW, H], DT, tag="e")
            nc.vector.tensor_tensor(
                out=e[:, 0:TL, :], in0=glo[:, :, scol0:scol0 + H],
                in1=psT[:, 0:TL * H].rearrange("p (t h) -> p t h", h=H),
                op=ALU.add)
            nc.vector.tensor_tensor(
                out=e[:, TL:TW, :], in0=ghi[:, :, scol0:scol0 + H],
                in1=psT[:, TL * H:TW * H].rearrange("p (t h) -> p t h", h=H),
                op=ALU.add)
            el = psm.tile([128, TW, H], DT, tag="el")
            nc.vector.scalar_tensor_tensor(
                out=el[:], in0=e[:], scalar=0.2, in1=e[:],
                op0=ALU.mult, op1=ALU.max)
            rhs = psm.tile([128, TW, ncol + H], BF, tag="rhs")
            nc.scalar.activation(rhs[:, :, ncol:ncol + H], el[:], AF.Exp)

            # msg = hp_src * exp (per head block), bf16
            dsub = ncol // H
            nc.vector.tensor_tensor(
                out=rhs[:, 0:TL, 0:ncol].rearrange("p t (h d) -> p t h d", d=dsub),
                in0=glo[:, :, 0:ncol].rearrange("p t (h d) -> p t h d", d=dsub),
                in1=rhs[:, 0:TL, ncol:ncol + H, None]
                    .to_broadcast([128, TL, H, dsub]),
                op=ALU.mult)
            nc.vector.tensor_tensor(
                out=rhs[:, TL:TW, 0:ncol].rearrange("p t (h d) -> p t h d", d=dsub),
                in0=ghi[:, :, 0:ncol].rearrange("p t (h d) -> p t h d", d=dsub),
                in1=rhs[:, TL:TW, ncol:ncol + H, None]
                    .to_broadcast([128, TH, H, dsub]),
                op=ALU.mult)

            # segment sum: accumulate [num | den] over the window's tiles
            psA = psp.tile([128, ncol + H], DT, tag="psA")
            for t in range(TW):
                P = pp.tile([128, 128], BF, tag="P")
                nc.vector.tensor_scalar(
                    out=P[:], in0=io[:], scalar1=tg[:, w * TW + t:w * TW + t + 1],
                    scalar2=None, op0=ALU.is_equal)
                nc.tensor.matmul(psA[:], lhsT=P[:], rhs=rhs[:, t, :],
                                 start=(t == 0), stop=(t == TW - 1))

            post_fn(w, psA)


def _build_l2(cfg):
    """Layer-0 edge phase: gathers from hpS (lo/hi split), t expanded; out h1."""
    W, NP, H = cfg["W"], cfg["NP"], cfg["H"]
    SPLIT, N = cfg["SPLIT"], cfg["N"]
    TL, TH = cfg["TL"], cfg["TH"]
    nc = bacc.Bacc(None, target_bir_lowering=False, num_swdge_queues=NQ)
    hpL = nc.dram_tensor("hpL", [SPLIT, 192], DT, kind="ExternalInput")
    hpH = nc.dram_tensor("hpH", [N - SPLIT, 192], DT, kind="ExternalInput")
    tS = nc.dram_tensor("tS", [NP, 64], DT, kind="ExternalInput")
    idxlo = nc.dram_tensor("idxlo", [128, W * TL * 8], I16, kind="ExternalInput")
    idxhi = nc.dram_tensor("idxhi", [128, W * TH * 8], I16, kind="ExternalInput")
    tg_d = nc.dram_tensor("tg", [128, W * (TL + TH)], DT, kind="ExternalInput")
    tgr_d = nc.dram_tensor("tgr", [1, W * (TL + TH) * 128], BF,
                           kind="ExternalInput")
    iota_d = nc.dram_tensor("iota", [128, 128], BF, kind="ExternalInput")
    iotac_d = nc.dram_tensor("iotac", [128, 1], DT, kind="ExternalInput")
    h1 = nc.dram_tensor("h1", [NP, 128], DT, kind="ExternalOutput")

    with TileContext(nc) as tc:
        with tc.tile_pool(name="post", bufs=3) as pq:
            def post(w, psA):
                # h0 = num / (den + 1e-16); h1 = elu(h0)
                dn = pq.tile([128, H], DT, tag="dn")
                nc.vector.tensor_scalar(out=dn[:], in0=psA[:, 128:128 + H],
                                        scalar1=1e-16, scalar2=None, op0=ALU.add)
                rc = pq.tile([128, H], DT, tag="rc")
                nc.vector.reciprocal(rc[:], dn[:])
                h0 = pq.tile([128, 128], DT, tag="h0")
                nc.vector.tensor_tensor(
                    out=h0[:].rearrange("p (h d) -> p h d", d=16),
                    in0=psA[:, 0:128].rearrange("p (h d) -> p h d", d=16),
                    in1=rc[:, :, None].to_broadcast([128, H, 16]),
                    op=ALU.mult)
                hm = pq.tile([128, 128], DT, tag="hm")
                nc.vector.tensor_scalar(out=hm[:], in0=h0[:], scalar1=0.0,
                                        scalar2=None, op0=ALU.min)
                he = pq.tile([128, 128], DT, tag="he")
                nc.scalar.activation(he[:], hm[:], AF.Exp)
                ho = pq.tile([128, 128], DT, tag="ho")
                nc.vector.scalar_tensor_tensor(
                    out=ho[:], in0=h0[:], scalar=0.0, in1=he[:],
                    op0=ALU.max, op1=ALU.add)
                nc.vector.tensor_scalar(out=ho[:], in0=ho[:], scalar1=1.0,
                                        scalar2=None, op0=ALU.subtract)
                nc.sync.dma_start(h1[w * 128:(w + 1) * 128, :], ho[:])

            _edge_phase(nc, tc, cfg, hpL, hpH, tS, idxlo, idxhi, tg_d, tgr_d,
                        iota_d, iotac_d, gcols=192, scol0=128, ncol=128,
                        post_fn=post)
    nc.compile()
    return nc


def _build_l3(cfg):
    """Layer 1: full projection of h1 -> hp1S, edge phase, log_softmax."""
    W, NP, NPF = cfg["W"], cfg["NP"], cfg["NPF"]
    SPLIT = cfg["SPLIT"]
    TL, TH = cfg["TL"], cfg["TH"]
    CH = NPF // 128
    B = 4
    nc = bacc.Bacc(None, target_bir_lowering=False, num_swdge_queues=NQ)
    h1T = nc.dram_tensor("h1T", [128, NPF], DT, kind="ExternalInput")
    h1To = nc.dram_tensor("h1To", [128, NP], DT, kind="ExternalInput")
    w1e = nc.dram_tensor("w1e", [128, 4], DT, kind="ExternalInput")
    idxlo = nc.dram_tensor("idxlo", [128, W * TL * 8], I16, kind="ExternalInput")
    idxhi = nc.dram_tensor("idxhi", [128, W * TH * 8], I16, kind="ExternalInput")
    tg_d = nc.dram_tensor("tg", [128, W * (TL + TH)], DT, kind="ExternalInput")
    tgr_d = nc.dram_tensor("tgr", [1, W * (TL + TH) * 128], BF,
                           kind="ExternalInput")
    iota_d = nc.dram_tensor("iota", [128, 128], BF, kind="ExternalInput")
    iotac_d = nc.dram_tensor("iotac", [128, 1], DT, kind="ExternalInput")
    outp = nc.dram_tensor("outp", [128, W, 2], DT, kind="ExternalOutput")

    hp1L = nc.dram_tensor("hp1L", [SPLIT, 64], DT)
    hp1H = nc.dram_tensor("hp1H", [NPF - SPLIT, 64], DT)
    t1S = nc.dram_tensor("t1S", [NP, 64], DT)

    with TileContext(nc) as tc:
        # (a) full projection hp1S rows = [hp1(2) | s1(1) | t1(1)]
        with (
            tc.tile_pool(name="pcs", bufs=1) as cs,
            tc.tile_pool(name="psb", bufs=3) as sb,
            tc.tile_pool(name="pps", bufs=2, space="PSUM") as psp,
        ):
            w1t = cs.tile([128, 4], DT)
            nc.sync.dma_start(w1t[:], w1e[:])
            for c0 in range(0, CH, B):
                nb = min(B, CH - c0)
                ht = sb.tile([128, B * 128], DT, tag="ht")
                nc.sync.dma_start(ht[:, 0:nb * 128],
                                  h1T[:, c0 * 128:(c0 + nb) * 128])
                ot = sb.tile([128, B, 4], DT, tag="ot")
                ps = psp.tile([128, B * 4], DT, tag="psa")
                for i in range(nb):
                    nc.tensor.matmul(ps[:, i * 4:(i + 1) * 4],
                                     lhsT=ht[:, i * 128:(i + 1) * 128],
                                     rhs=w1t[:], start=True, stop=True)
                nc.vector.tensor_copy(
                    ot[:, 0:nb, :],
                    ps[:, 0:nb * 4].rearrange("p (c d) -> p c d", d=4))
                r0, r1 = c0 * 128, (c0 + nb) * 128
                eng = nc.sync if (c0 // B) % 2 == 0 else nc.scalar
                if r1 <= SPLIT:
                    eng.dma_start(
                        hp1L[r0:r1, 0:4].rearrange("(c p) d -> p c d", p=128),
                        ot[:, 0:nb, :])
                elif r0 >= SPLIT:
                    eng.dma_start(
                        hp1H[r0 - SPLIT:r1 - SPLIT, 0:4]
                        .rearrange("(c p) d -> p c d", p=128),
                        ot[:, 0:nb, :])
                else:
                    for i in range(nb):
                        rr0 = r0 + i * 128
                        if rr0 + 128 <= SPLIT:
                            eng.dma_start(hp1L[rr0:rr0 + 128, 0:4], ot[:, i, :])
                        elif rr0 >= SPLIT:
                            eng.dma_start(
                                hp1H[rr0 - SPLIT:rr0 - SPLIT + 128, 0:4],
                                ot[:, i, :])
                        else:
                            k = SPLIT - rr0
                            eng.dma_start(hp1L[rr0:SPLIT, 0:4], ot[0:k, i, :])
                            eng.dma_start(hp1H[0:rr0 + 128 - SPLIT, 0:4],
                                          ot[k:128, i, :])

            # (b) t1 for own nodes -> t1S col 0:1
            for c0 in range(0, W, B):
                nb = min(B, W - c0)
                ht = sb.tile([128, B * 128], DT, tag="ht")
                nc.sync.dma_start(ht[:, 0:nb * 128],
                                  h1To[:, c0 * 128:(c0 + nb) * 128])
                ot = sb.tile([128, B, 4], DT, tag="ot")
                ps = psp.tile([128, B * 4], DT, tag="psa")
                for i in range(nb):
                    nc.tensor.matmul(ps[:, i * 4:(i + 1) * 4],
                                     lhsT=ht[:, i * 128:(i + 1) * 128],
                                     rhs=w1t[:], start=True, stop=True)
                nc.vector.tensor_copy(
                    ot[:, 0:nb, :],
                    ps[:, 0:nb * 4].rearrange("p (c d) -> p c d", d=4))
                r0, r1 = c0 * 128, (c0 + nb) * 128
                nc.scalar.dma_start(
                    t1S[r0:r1, 0:1].rearrange("(c p) d -> p c d", p=128),
                    ot[:, 0:nb, 3:4])

        # (c) edge phase; accumulate per-window logits
        with tc.tile_pool(name="post", bufs=1) as pq:
            acc = pq.tile([128, W, 3], DT)

            def post(w, psA):
                nc.vector.tensor_copy(acc[:, w, :], psA[:, 0:3])

            cfg3 = dict(cfg)
            cfg3["H"] = 1
            _edge_phase(nc, tc, cfg3, hp1L, hp1H, t1S, idxlo, idxhi, tg_d,
                        tgr_d, iota_d, iotac_d, gcols=64, scol0=2, ncol=2,
                        post_fn=post)

            # (d) batched normalize + log_softmax
            dn = pq.tile([128, W], DT)
            nc.vector.tensor_scalar(out=dn[:], in0=acc[:, :, 2], scalar1=1e-16,
                                    scalar2=None, op0=ALU.add)
            rc = pq.tile([128, W], DT)
            nc.vector.reciprocal(rc[:], dn[:])
            lg = pq.tile([128, W, 2], DT)
            nc.vector.tensor_tensor(out=lg[:], in0=acc[:, :, 0:2],
                                    in1=rc[:, :, None].to_broadcast([128, W, 2]),
                                    op=ALU.mult)
            mx = pq.tile([128, W], DT)
            nc.vector.tensor_reduce(out=mx[:], in_=lg[:],
                                    axis=mybir.AxisListType.X, op=ALU.max)
            dd = pq.tile([128, W, 2], DT)
            nc.vector.tensor_tensor(out=dd[:], in0=lg[:],
                                    in1=mx[:, :, None].to_broadcast([128, W, 2]),
                                    op=ALU.subtract)
            e2 = pq.tile([128, W, 2], DT)
            nc.scalar.activation(e2[:], dd[:], AF.Exp)
            se = pq.tile([128, W], DT)
            nc.vector.tensor_reduce(out=se[:], in_=e2[:],
                                    axis=mybir.AxisListType.X, op=ALU.add)
            ls = pq.tile([128, W], DT)
            nc.scalar.activation(ls[:], se[:], AF.Ln)
            ov = pq.tile([128, W, 2], DT)
            nc.vector.tensor_tensor(out=ov[:], in0=dd[:],
                                    in1=ls[:, :, None].to_broadcast([128, W, 2]),
                                    op=ALU.subtract)
            nc.sync.dma_start(outp[:], ov[:])
    nc.compile()
    return nc


# --------------------------------------------------------------------------
# host-side prep (index/layout only)
# --------------------------------------------------------------------------

def _prep_edges(src, trg, cfg):
    """Partition+sort edges by destination; build per-core gather index and
    one-hot-builder arrays. Returns (TL, TH, per_core list of dicts)."""
    N, NPC, W, SPLIT = cfg["N"], cfg["NPC"], cfg["W"], cfg["SPLIT"]
    src = np.asarray(src).astype(np.int64)
    trg = np.asarray(trg).astype(np.int64)

    cores = []
    max_lo = 1
    max_hi = 1
    for c in range(N_CORES):
        m = (trg >= c * NPC) & (trg < (c + 1) * NPC)
        es, et = src[m], trg[m] - c * NPC
        o = np.argsort(et, kind="stable")
        es, et = es[o], et[o]
        bounds = np.searchsorted(et, np.arange(W + 1) * 128)
        wins = []
        for w in range(W):
            ws, wt = es[bounds[w]:bounds[w + 1]], et[bounds[w]:bounds[w + 1]]
            lo = ws < SPLIT
            wins.append((ws[lo], wt[lo], ws[~lo] - SPLIT, wt[~lo]))
            max_lo = max(max_lo, int(lo.sum()))
            max_hi = max(max_hi, int((~lo).sum()))
        cores.append(wins)

    TL = (max_lo + 127) // 128
    TH = (max_hi + 127) // 128
    TW = TL + TH

    per_core = []
    for c in range(N_CORES):
        idxlo = np.zeros((W, TL * 128), np.int64)
        idxhi = np.zeros((W, TH * 128), np.int64)
        trgl = np.full((W, TW * 128), -1.0, np.float32)
        for w, (slo, tlo, shi, thi) in enumerate(cores[c]):
            nlo, nhi = len(slo), len(shi)
            idxlo[w, :nlo] = slo
            trgl[w, :nlo] = tlo - 128 * w
            idxhi[w, :nhi] = shi
            trgl[w, TL * 128:TL * 128 + nhi] = thi - 128 * w
        per_core.append(dict(
            idxlo=np.concatenate([_wrap_idx(idxlo[w]) for w in range(W)], axis=1),
            idxhi=np.concatenate([_wrap_idx(idxhi[w]) for w in range(W)], axis=1),
            tg=np.ascontiguousarray(
                np.stack([trgl[w].reshape(TW, 128).T for w in range(W)], axis=1)
                .reshape(128, W * TW)),
            tgr=np.ascontiguousarray(trgl.reshape(1, W * TW * 128)).astype(NPBF),
        ))
    return TL, TH, per_core


_NC_CACHE = {}


def _cached(key, build, cfg):
    if key not in _NC_CACHE:
        _NC_CACHE[key] = build(cfg)
    return _NC_CACHE[key]


def _run(nc, in_maps, **kw):
    return run_bass_kernel_spmd(nc, in_maps, list(range(N_CORES)), **kw)


def kernel(static_emb, dyn0, dyn1, src_indices, trg_indices,
           w0, asrc0, atrg0, w1, asrc1, atrg1, _cfg=None, _runner=None):
    cfg = dict(_cfg_full() if _cfg is None else _cfg)
    N, NPC, W, NP, NPF = cfg["N"], cfg["NPC"], cfg["W"], cfg["NP"], cfg["NPF"]
    SPLIT = cfg["SPLIT"]
    run = _runner if _runner is not None else _run

    f32 = np.float32
    emb = np.concatenate([np.asarray(dyn0, f32), np.asarray(dyn1, f32),
                          np.asarray(static_emb, f32)], axis=1)  # [N, 128]
    embT = np.ascontiguousarray(emb.T)  # [128, N]

    w0 = np.asarray(w0, f32)
    w0e = np.zeros((128, 144), f32)
    w0e[:, :128] = w0.transpose(1, 0, 2).reshape(128, 128)
    w0e[:, 128:136] = np.einsum("hfd,hd->fh", w0, np.asarray(asrc0, f32)[:, :, 0])
    w0e[:, 136:144] = np.einsum("hfd,hd->fh", w0, np.asarray(atrg0, f32)[:, :, 0])

    w1 = np.asarray(w1, f32)
    w1e = np.zeros((128, 4), f32)
    w1e[:, 0:2] = w1[0]
    w1e[:, 2] = w1[0] @ np.asarray(asrc1, f32)[0, :, 0]
    w1e[:, 3] = w1[0] @ np.asarray(atrg1, f32)[0, :, 0]

    TL, TH, eprep = _prep_edges(src_indices, trg_indices, cfg)
    cfg["TL"], cfg["TH"] = TL, TH
    cfg["H"] = 8

    iota = np.broadcast_to(np.arange(128, dtype=f32), (128, 128)).astype(NPBF)
    iotac = np.arange(128, dtype=f32).reshape(128, 1)

    # ---- L1: sharded projection -----------------------------------------
    nc1 = _cached(("l1", NP), _build_l1, cfg)
    in1 = []
    for c in range(N_CORES):
        eo = np.zeros((128, NP), f32)
        eo[:, :NPC] = embT[:, c * NPC:(c + 1) * NPC]
        in1.append(dict(embT=eo, w0e=w0e))
    r1 = run(nc1, in1)

    hpS = np.concatenate([r1.results[c]["hpS"][:NPC] for c in range(N_CORES)])
    hpL = np.ascontiguousarray(hpS[:SPLIT])
    hpH = np.ascontiguousarray(hpS[SPLIT:])

    # ---- L2: layer-0 edge phase -----------------------------------------
    nc2 = _cached(("l2", NP, TL, TH), _build_l2, cfg)
    in2 = []
    for c in range(N_CORES):
        in2.append(dict(hpL=hpL, hpH=hpH, tS=r1.results[c]["tS"],
                        idxlo=eprep[c]["idxlo"], idxhi=eprep[c]["idxhi"],
                        tg=eprep[c]["tg"], tgr=eprep[c]["tgr"],
                        iota=iota, iotac=iotac))
    r2 = run(nc2, in2)

    h1 = np.zeros((NPF, 128), f32)
    for c in range(N_CORES):
        h1[c * NPC:(c + 1) * NPC] = r2.results[c]["h1"][:NPC]
    h1T = np.ascontiguousarray(h1.T)  # [128, NPF]

    # ---- L3: layer 1 + log_softmax --------------------------------------
    nc3 = _cached(("l3", NP, TL, TH), _build_l3, cfg)
    in3 = []
    for c in range(N_CORES):
        ho = np.zeros((128, NP), f32)
        ho[:, :NPC] = h1T[:, c * NPC:(c + 1) * NPC]
        in3.append(dict(h1T=h1T, h1To=ho, w1e=w1e,
                        idxlo=eprep[c]["idxlo"], idxhi=eprep[c]["idxhi"],
                        tg=eprep[c]["tg"], tgr=eprep[c]["tgr"],
                        iota=iota, iotac=iotac))
    r3 = run(nc3, in3)

    out = np.zeros((N, 2), f32)
    for c in range(N_CORES):
        o = r3.results[c]["outp"]  # [128, W, 2]; node = w*128 + p (local)
        loc = np.transpose(o, (1, 0, 2)).reshape(NP, 2)
        out[c * NPC:(c + 1) * NPC] = loc[:NPC]
    return out


# revision 7
# speedup vs baseline: 3.4685x; 1.2701x over previous
"""DenseSparseGAT (2-layer SpGAT, N=50000, E=800000) on 8 trn2 NeuronCores.

Strategy (graph/data parallel, per the sharding hint):
  - Nodes partitioned into 8 contiguous blocks of 6250; edges assigned to the
    core owning their destination (trg) node, sorted by trg within a core.
  - Per-edge source-node features fetched with SWDGE dma_gather from an
    HBM-resident projected-feature table (full table per core = replicated
    halo). Gathers are issued per 128-node window on rotating SWDGE queues
    (4 queues) so descriptor rings drain in parallel.
  - Destination-side attention terms are NOT gathered: within a window the
    trg values span 128 nodes, so t[trg] is expanded with a one-hot matmul
    (P_T[node, edge] built from a broadcast + is_equal, then PE matmul
    against the window's t rows).
  - segment_sum is a one-hot matmul into PSUM per window: lhsT = P[edge,
    node] one-hot (built with one is_equal per 128-edge tile), rhs =
    [msg | exp] packed bf16.
  - Softmax max-subtraction is skipped: alpha = exp(e)/sum(exp(e)) is
    shift-invariant and the scores are small enough that fp32 exp cannot
    overflow here.
  - Three SPMD launches: L1 projection (sharded), L2 layer-0 edge phase,
    L3 layer-1 (projection + edge phase + log_softmax). Host only does
    layout work (concat/transpose/sort/pad) between launches.
"""

import numpy as np

from concourse import bacc, bass, mybir
from concourse.tile import TileContext
from concourse.bass_utils import run_bass_kernel_spmd

DT = mybir.dt.float32
BF = mybir.dt.bfloat16
I16 = mybir.dt.int16
AF = mybir.ActivationFunctionType
ALU = mybir.AluOpType
NPBF = mybir.dt.np(BF)

N_CORES = 8
NQ = 4  # SWDGE queues


def _cfg_full():
    N = 50000
    NPC = N // N_CORES            # 6250 nodes per core
    W = (NPC + 127) // 128        # 49 windows
    NP = W * 128                  # 6272 padded nodes per core
    NPF = ((N + 127) // 128) * 128  # 50048 padded total nodes
    return dict(N=N, NPC=NPC, W=W, NP=NP, NPF=NPF, SPLIT=N // 2, H=8)


def _wrap_idx(a):
    """[num] ints -> dma_gather idx layout [128, num//16] int16 (replicated)."""
    num = a.shape[0]
    assert num % 16 == 0
    blk = np.ascontiguousarray(a.reshape(num // 16, 16).T.astype(np.int16))
    return np.tile(blk, (8, 1))


# --------------------------------------------------------------------------
# kernel builders
# --------------------------------------------------------------------------

def _build_l1(cfg):
    """Projection: hpS[n, :144] = [hp(128) | s(8) | t(8)], tS[n, :8] = t."""
    W, NP = cfg["W"], cfg["NP"]
    nc = bacc.Bacc(None, target_bir_lowering=False)
    embT = nc.dram_tensor("embT", [128, NP], DT, kind="ExternalInput")
    w0e = nc.dram_tensor("w0e", [128, 144], DT, kind="ExternalInput")
    hpS = nc.dram_tensor("hpS", [NP, 256], BF, kind="ExternalOutput")
    tS = nc.dram_tensor("tS", [NP, 64], DT, kind="ExternalOutput")

    B = 4  # chunks per DMA batch
    with TileContext(nc) as tc:
        with (
            tc.tile_pool(name="cs", bufs=1) as cs,
            tc.tile_pool(name="sb", bufs=3) as sb,
            tc.tile_pool(name="ps", bufs=2, space="PSUM") as psp,
        ):
            w0t = cs.tile([128, 144], DT)
            nc.sync.dma_start(w0t[:], w0e[:])
            for c0 in range(0, W, B):
                nb = min(B, W - c0)
                et = sb.tile([128, B * 128], DT, tag="et")
                nc.sync.dma_start(et[:, 0:nb * 128],
                                  embT[:, c0 * 128:(c0 + nb) * 128])
                ot = sb.tile([128, B, 16], DT, tag="ot")
                otb = sb.tile([128, B, 128], BF, tag="otb")
                for i in range(nb):
                    ps = psp.tile([128, 144], DT)
                    nc.tensor.matmul(ps[:], lhsT=et[:, i * 128:(i + 1) * 128],
                                     rhs=w0t[:], start=True, stop=True)
                    nc.vector.tensor_copy(ot[:, i, :], ps[:, 128:144])
                    nc.vector.tensor_copy(otb[:, i, :], ps[:, 0:128])
                r0, r1 = c0 * 128, (c0 + nb) * 128
                nc.sync.dma_start(
                    hpS[r0:r1, 0:128].rearrange("(c p) d -> p c d", p=128),
                    otb[:, 0:nb, :])
                nc.scalar.dma_start(
                    hpS[r0:r1, 128:144].bitcast(DT)
                    .rearrange("(c p) d -> p c d", p=128),
                    ot[:, 0:nb, 0:8])
                nc.scalar.dma_start(
                    tS[r0:r1, 0:8].rearrange("(c p) d -> p c d", p=128),
                    ot[:, 0:nb, 8:16])
    nc.compile()
    return nc


def _edge_phase(nc, tc, cfg, src_lo, src_hi, tdram, idxlo, idxhi, tg_d, tgr_d,
                iota_d, iotac_d, gdt, gcols, s_view, ncol, post_fn):
    """Shared edge phase for one GAT layer.

    src_lo/src_hi: DRAM tables gathered by (split) src index; fp32 rows of
      gcols (msg features at [0:ncol], s at [scol0:scol0+H]).
    tdram: fp32 [NP, 64] table, t at cols [0:H], read per window (no gather).
    """
    W, H = cfg["W"], cfg["H"]
    W_RUN = cfg.get("W_RUN", W)
    TL, TH = cfg["TL"], cfg["TH"]
    TW = TL + TH
    EW = TW * 128  # edge slots per window
    qctr = [0]

    def gather(dst, srcd, idxs, col0, ntiles):
        for j in range(0, ntiles, 8):
            k = min(8, ntiles - j)
            nc.gpsimd.dma_gather(
                dst[:, j:j + k, :], srcd[:],
                idxs[:, col0 + j * 8:col0 + (j + k) * 8],
                k * 128, k * 128, gcols, queue_num=qctr[0] % NQ)
            qctr[0] += 1

    with (
        tc.tile_pool(name="ecs", bufs=1) as cs,
        tc.tile_pool(name="eg", bufs=3) as pg,
        tc.tile_pool(name="esm", bufs=2) as psm,
        tc.tile_pool(name="epp", bufs=2) as pp,
        tc.tile_pool(name="eps", bufs=2, space="PSUM") as psp,
    ):
        ilo = cs.tile([128, W * TL * 8], I16)
        nc.sync.dma_start(ilo[:], idxlo[:])
        ihi = cs.tile([128, W * TH * 8], I16)
        nc.sync.dma_start(ihi[:], idxhi[:])
        tg = cs.tile([128, W * TW], DT)
        nc.sync.dma_start(tg[:], tg_d[:])
        io = cs.tile([128, 128], BF)
        nc.sync.dma_start(io[:], iota_d[:])
        ioc = cs.tile([128, 1], DT)
        nc.sync.dma_start(ioc[:], iotac_d[:])
        ones = cs.tile([1, 128], BF)
        nc.gpsimd.memset(ones[:], 1.0)

        for w in range(W_RUN):
            glo = pg.tile([128, TL, gcols], gdt, tag="glo")
            gather(glo, src_lo, ilo, w * TL * 8, TL)
            ghi = pg.tile([128, TH, gcols], gdt, tag="ghi")
            gather(ghi, src_hi, ihi, w * TH * 8, TH)

            # window t rows + trgl broadcast row
            twf = psm.tile([128, H], DT, tag="twf")
            nc.scalar.dma_start(twf[:], tdram[w * 128:(w + 1) * 128, 0:H])
            twb = psm.tile([128, H], BF, tag="twb")
            nc.vector.tensor_copy(twb[:], twf[:])
            tgr = psm.tile([1, EW], BF, tag="tgr")
            nc.scalar.dma_start(tgr[:], tgr_d[:, w * EW:(w + 1) * EW])

            # P_T[node, edge] one-hot: broadcast trgl row, compare to iota col
            PT = psm.tile([128, EW], BF, tag="PT")
            for o in range(0, EW, 512):
                S = min(512, EW - o)
                psb = psp.tile([128, 512], DT, tag="psb")
                nc.tensor.matmul(psb[:, 0:S], lhsT=ones[:], rhs=tgr[:, o:o + S],
                                 start=True, stop=True)
                nc.vector.tensor_scalar(
                    out=PT[:, o:o + S], in0=psb[:, 0:S], scalar1=ioc[:],
                    scalar2=None, op0=ALU.is_equal)

            # t_edge[edge, h] = P_T^T @ t_win, per 128-edge tile into one bank
            psT = psp.tile([128, TW * H], DT, tag="psT")
            for t in range(TW):
                nc.tensor.matmul(psT[:, t * H:(t + 1) * H],
                                 lhsT=PT[:, t * 128:(t + 1) * 128],
                                 rhs=twb[:], start=True, stop=True)

            # e = leaky_relu(s_src + t_trg, 0.2); rhs = [msg | exp] bf16
            e = psm.tile([128, TW, H], DT, tag="e")
            nc.vector.tensor_tensor(
                out=e[:, 0:TL, :], in0=s_view(glo),
                in1=psT[:, 0:TL * H].rearrange("p (t h) -> p t h", h=H),
                op=ALU.add)
            nc.vector.tensor_tensor(
                out=e[:, TL:TW, :], in0=s_view(ghi),
                in1=psT[:, TL * H:TW * H].rearrange("p (t h) -> p t h", h=H),
                op=ALU.add)
            el = psm.tile([128, TW, H], DT, tag="el")
            nc.vector.scalar_tensor_tensor(
                out=el[:], in0=e[:], scalar=0.2, in1=e[:],
                op0=ALU.mult, op1=ALU.max)
            rhs = psm.tile([128, TW, ncol + H], BF, tag="rhs")
            nc.scalar.activation(rhs[:, :, ncol:ncol + H], el[:], AF.Exp)

            # msg = hp_src * exp (per head block), bf16
            dsub = ncol // H
            nc.vector.tensor_tensor(
                out=rhs[:, 0:TL, 0:ncol].rearrange("p t (h d) -> p t h d", d=dsub),
                in0=glo[:, :, 0:ncol].rearrange("p t (h d) -> p t h d", d=dsub),
                in1=rhs[:, 0:TL, ncol:ncol + H, None]
                    .to_broadcast([128, TL, H, dsub]),
                op=ALU.mult)
            nc.vector.tensor_tensor(
                out=rhs[:, TL:TW, 0:ncol].rearrange("p t (h d) -> p t h d", d=dsub),
                in0=ghi[:, :, 0:ncol].rearrange("p t (h d) -> p t h d", d=dsub),
                in1=rhs[:, TL:TW, ncol:ncol + H, None]
                    .to_broadcast([128, TH, H, dsub]),
                op=ALU.mult)

            # segment sum: accumulate [num | den] over the window's tiles
            Pall = pp.tile([128, TW, 128], BF, tag="Pall")
            nc.vector.tensor_tensor(
                out=Pall[:],
                in0=io[:, None, :].to_broadcast([128, TW, 128]),
                in1=tg[:, w * TW:(w + 1) * TW, None].to_broadcast([128, TW, 128]),
                op=ALU.is_equal)
            psA = psp.tile([128, ncol + H], DT, tag="psA")
            for t in range(TW):
                nc.tensor.matmul(psA[:], lhsT=Pall[:, t, :], rhs=rhs[:, t, :],
                                 start=(t == 0), stop=(t == TW - 1))

            post_fn(w, psA)


def _build_l2(cfg):
    """Layer-0 edge phase: gathers from hpS (lo/hi split), t expanded; out h1."""
    W, NP, H = cfg["W"], cfg["NP"], cfg["H"]
    SPLIT, N = cfg["SPLIT"], cfg["N"]
    TL, TH = cfg["TL"], cfg["TH"]
    nc = bacc.Bacc(None, target_bir_lowering=False, num_swdge_queues=NQ)
    hpL = nc.dram_tensor("hpL", [SPLIT, 256], BF, kind="ExternalInput")
    hpH = nc.dram_tensor("hpH", [N - SPLIT, 256], BF, kind="ExternalInput")
    tS = nc.dram_tensor("tS", [NP, 64], DT, kind="ExternalInput")
    idxlo = nc.dram_tensor("idxlo", [128, W * TL * 8], I16, kind="ExternalInput")
    idxhi = nc.dram_tensor("idxhi", [128, W * TH * 8], I16, kind="ExternalInput")
    tg_d = nc.dram_tensor("tg", [128, W * (TL + TH)], DT, kind="ExternalInput")
    tgr_d = nc.dram_tensor("tgr", [1, W * (TL + TH) * 128], BF,
                           kind="ExternalInput")
    iota_d = nc.dram_tensor("iota", [128, 128], BF, kind="ExternalInput")
    iotac_d = nc.dram_tensor("iotac", [128, 1], DT, kind="ExternalInput")
    h1 = nc.dram_tensor("h1", [NP, 128], DT, kind="ExternalOutput")

    with TileContext(nc) as tc:
        with tc.tile_pool(name="post", bufs=3) as pq:
            def post(w, psA):
                # h0 = num / (den + 1e-16); h1 = elu(h0)
                dn = pq.tile([128, H], DT, tag="dn")
                nc.vector.tensor_scalar(out=dn[:], in0=psA[:, 128:128 + H],
                                        scalar1=1e-16, scalar2=None, op0=ALU.add)
                rc = pq.tile([128, H], DT, tag="rc")
                nc.vector.reciprocal(rc[:], dn[:])
                h0 = pq.tile([128, 128], DT, tag="h0")
                nc.vector.tensor_tensor(
                    out=h0[:].rearrange("p (h d) -> p h d", d=16),
                    in0=psA[:, 0:128].rearrange("p (h d) -> p h d", d=16),
                    in1=rc[:, :, None].to_broadcast([128, H, 16]),
                    op=ALU.mult)
                hm = pq.tile([128, 128], DT, tag="hm")
                nc.vector.tensor_scalar(out=hm[:], in0=h0[:], scalar1=0.0,
                                        scalar2=None, op0=ALU.min)
                he = pq.tile([128, 128], DT, tag="he")
                nc.scalar.activation(he[:], hm[:], AF.Exp)
                ho = pq.tile([128, 128], DT, tag="ho")
                nc.vector.scalar_tensor_tensor(
                    out=ho[:], in0=h0[:], scalar=0.0, in1=he[:],
                    op0=ALU.max, op1=ALU.add)
                nc.vector.tensor_scalar(out=ho[:], in0=ho[:], scalar1=1.0,
                                        scalar2=None, op0=ALU.subtract)
                nc.sync.dma_start(h1[w * 128:(w + 1) * 128, :], ho[:])

            _edge_phase(nc, tc, cfg, hpL, hpH, tS, idxlo, idxhi, tg_d, tgr_d,
                        iota_d, iotac_d, gdt=BF, gcols=256,
                        s_view=lambda g: g[:, :, 128:144].bitcast(DT),
                        ncol=128, post_fn=post)
    nc.compile()
    return nc


def _build_l3(cfg):
    """Layer 1: full projection of h1 -> hp1S, edge phase, log_softmax."""
    W, NP, NPF = cfg["W"], cfg["NP"], cfg["NPF"]
    SPLIT = cfg["SPLIT"]
    TL, TH = cfg["TL"], cfg["TH"]
    CH = NPF // 128
    B = 4
    nc = bacc.Bacc(None, target_bir_lowering=False, num_swdge_queues=NQ)
    h1T = nc.dram_tensor("h1T", [128, NPF], DT, kind="ExternalInput")
    h1To = nc.dram_tensor("h1To", [128, NP], DT, kind="ExternalInput")
    w1e = nc.dram_tensor("w1e", [128, 4], DT, kind="ExternalInput")
    idxlo = nc.dram_tensor("idxlo", [128, W * TL * 8], I16, kind="ExternalInput")
    idxhi = nc.dram_tensor("idxhi", [128, W * TH * 8], I16, kind="ExternalInput")
    tg_d = nc.dram_tensor("tg", [128, W * (TL + TH)], DT, kind="ExternalInput")
    tgr_d = nc.dram_tensor("tgr", [1, W * (TL + TH) * 128], BF,
                           kind="ExternalInput")
    iota_d = nc.dram_tensor("iota", [128, 128], BF, kind="ExternalInput")
    iotac_d = nc.dram_tensor("iotac", [128, 1], DT, kind="ExternalInput")
    outp = nc.dram_tensor("outp", [128, W, 2], DT, kind="ExternalOutput")

    hp1L = nc.dram_tensor("hp1L", [SPLIT, 64], DT)
    hp1H = nc.dram_tensor("hp1H", [NPF - SPLIT, 64], DT)
    t1S = nc.dram_tensor("t1S", [NP, 64], DT)

    with TileContext(nc) as tc:
        # (a) full projection hp1S rows = [hp1(2) | s1(1) | t1(1)]
        with (
            tc.tile_pool(name="pcs", bufs=1) as cs,
            tc.tile_pool(name="psb", bufs=3) as sb,
            tc.tile_pool(name="pps", bufs=2, space="PSUM") as psp,
        ):
            w1t = cs.tile([128, 4], DT)
            nc.sync.dma_start(w1t[:], w1e[:])
            for c0 in range(0, CH, B):
                nb = min(B, CH - c0)
                ht = sb.tile([128, B * 128], DT, tag="ht")
                nc.sync.dma_start(ht[:, 0:nb * 128],
                                  h1T[:, c0 * 128:(c0 + nb) * 128])
                ot = sb.tile([128, B, 4], DT, tag="ot")
                ps = psp.tile([128, B * 4], DT, tag="psa")
                for i in range(nb):
                    nc.tensor.matmul(ps[:, i * 4:(i + 1) * 4],
                                     lhsT=ht[:, i * 128:(i + 1) * 128],
                                     rhs=w1t[:], start=True, stop=True)
                nc.vector.tensor_copy(
                    ot[:, 0:nb, :],
                    ps[:, 0:nb * 4].rearrange("p (c d) -> p c d", d=4))
                r0, r1 = c0 * 128, (c0 + nb) * 128
                eng = nc.sync if (c0 // B) % 2 == 0 else nc.scalar
                if r1 <= SPLIT:
                    eng.dma_start(
                        hp1L[r0:r1, 0:4].rearrange("(c p) d -> p c d", p=128),
                        ot[:, 0:nb, :])
                elif r0 >= SPLIT:
                    eng.dma_start(
                        hp1H[r0 - SPLIT:r1 - SPLIT, 0:4]
                        .rearrange("(c p) d -> p c d", p=128),
                        ot[:, 0:nb, :])
                else:
                    for i in range(nb):
                        rr0 = r0 + i * 128
                        if rr0 + 128 <= SPLIT:
                            eng.dma_start(hp1L[rr0:rr0 + 128, 0:4], ot[:, i, :])
                        elif rr0 >= SPLIT:
                            eng.dma_start(
                                hp1H[rr0 - SPLIT:rr0 - SPLIT + 128, 0:4],
                                ot[:, i, :])
                        else:
                            k = SPLIT - rr0
                            eng.dma_start(hp1L[rr0:SPLIT, 0:4], ot[0:k, i, :])
                            eng.dma_start(hp1H[0:rr0 + 128 - SPLIT, 0:4],
                                          ot[k:128, i, :])

            # (b) t1 for own nodes -> t1S col 0:1
            for c0 in range(0, W, B):
                nb = min(B, W - c0)
                ht = sb.tile([128, B * 128], DT, tag="ht")
                nc.sync.dma_start(ht[:, 0:nb * 128],
                                  h1To[:, c0 * 128:(c0 + nb) * 128])
                ot = sb.tile([128, B, 4], DT, tag="ot")
                ps = psp.tile([128, B * 4], DT, tag="psa")
                for i in range(nb):
                    nc.tensor.matmul(ps[:, i * 4:(i + 1) * 4],
                                     lhsT=ht[:, i * 128:(i + 1) * 128],
                                     rhs=w1t[:], start=True, stop=True)
                nc.vector.tensor_copy(
                    ot[:, 0:nb, :],
                    ps[:, 0:nb * 4].rearrange("p (c d) -> p c d", d=4))
                r0, r1 = c0 * 128, (c0 + nb) * 128
                nc.scalar.dma_start(
                    t1S[r0:r1, 0:1].rearrange("(c p) d -> p c d", p=128),
                    ot[:, 0:nb, 3:4])

        # (c) edge phase; accumulate per-window logits
        with tc.tile_pool(name="post", bufs=1) as pq:
            acc = pq.tile([128, W, 3], DT)

            def post(w, psA):
                nc.vector.tensor_copy(acc[:, w, :], psA[:, 0:3])

            cfg3 = dict(cfg)
            cfg3["H"] = 1
            _edge_phase(nc, tc, cfg3, hp1L, hp1H, t1S, idxlo, idxhi, tg_d,
                        tgr_d, iota_d, iotac_d, gdt=DT, gcols=64,
                        s_view=lambda g: g[:, :, 2:3], ncol=2, post_fn=post)

            # (d) batched normalize + log_softmax
            dn = pq.tile([128, W], DT)
            nc.vector.tensor_scalar(out=dn[:], in0=acc[:, :, 2], scalar1=1e-16,
                                    scalar2=None, op0=ALU.add)
            rc = pq.tile([128, W], DT)
            nc.vector.reciprocal(rc[:], dn[:])
            lg = pq.tile([128, W, 2], DT)
            nc.vector.tensor_tensor(out=lg[:], in0=acc[:, :, 0:2],
                                    in1=rc[:, :, None].to_broadcast([128, W, 2]),
                                    op=ALU.mult)
            mx = pq.tile([128, W], DT)
            nc.vector.tensor_reduce(out=mx[:], in_=lg[:],
                                    axis=mybir.AxisListType.X, op=ALU.max)
            dd = pq.tile([128, W, 2], DT)
            nc.vector.tensor_tensor(out=dd[:], in0=lg[:],
                                    in1=mx[:, :, None].to_broadcast([128, W, 2]),
                                    op=ALU.subtract)
            e2 = pq.tile([128, W, 2], DT)
            nc.scalar.activation(e2[:], dd[:], AF.Exp)
            se = pq.tile([128, W], DT)
            nc.vector.tensor_reduce(out=se[:], in_=e2[:],
                                    axis=mybir.AxisListType.X, op=ALU.add)
            ls = pq.tile([128, W], DT)
            nc.scalar.activation(ls[:], se[:], AF.Ln)
            ov = pq.tile([128, W, 2], DT)
            nc.vector.tensor_tensor(out=ov[:], in0=dd[:],
                                    in1=ls[:, :, None].to_broadcast([128, W, 2]),
                                    op=ALU.subtract)
            nc.sync.dma_start(outp[:], ov[:])
    nc.compile()
    return nc


# --------------------------------------------------------------------------
# host-side prep (index/layout only)
# --------------------------------------------------------------------------

def _prep_edges(src, trg, cfg):
    """Partition+sort edges by destination; build per-core gather index and
    one-hot-builder arrays. Returns (TL, TH, per_core list of dicts)."""
    N, NPC, W, SPLIT = cfg["N"], cfg["NPC"], cfg["W"], cfg["SPLIT"]
    src = np.asarray(src).astype(np.int64)
    trg = np.asarray(trg).astype(np.int64)

    cores = []
    max_lo = 1
    max_hi = 1
    for c in range(N_CORES):
        m = (trg >= c * NPC) & (trg < (c + 1) * NPC)
        es, et = src[m], trg[m] - c * NPC
        o = np.argsort(et, kind="stable")
        es, et = es[o], et[o]
        bounds = np.searchsorted(et, np.arange(W + 1) * 128)
        wins = []
        for w in range(W):
            ws, wt = es[bounds[w]:bounds[w + 1]], et[bounds[w]:bounds[w + 1]]
            lo = ws < SPLIT
            wins.append((ws[lo], wt[lo], ws[~lo] - SPLIT, wt[~lo]))
            max_lo = max(max_lo, int(lo.sum()))
            max_hi = max(max_hi, int((~lo).sum()))
        cores.append(wins)

    TL = (max_lo + 127) // 128
    TH = (max_hi + 127) // 128
    TW = TL + TH

    per_core = []
    for c in range(N_CORES):
        idxlo = np.zeros((W, TL * 128), np.int64)
        idxhi = np.zeros((W, TH * 128), np.int64)
        trgl = np.full((W, TW * 128), -1.0, np.float32)
        for w, (slo, tlo, shi, thi) in enumerate(cores[c]):
            nlo, nhi = len(slo), len(shi)
            idxlo[w, :nlo] = slo
            trgl[w, :nlo] = tlo - 128 * w
            idxhi[w, :nhi] = shi
            trgl[w, TL * 128:TL * 128 + nhi] = thi - 128 * w
        per_core.append(dict(
            idxlo=np.concatenate([_wrap_idx(idxlo[w]) for w in range(W)], axis=1),
            idxhi=np.concatenate([_wrap_idx(idxhi[w]) for w in range(W)], axis=1),
            tg=np.ascontiguousarray(
                np.stack([trgl[w].reshape(TW, 128).T for w in range(W)], axis=1)
                .reshape(128, W * TW)),
            tgr=np.ascontiguousarray(trgl.reshape(1, W * TW * 128)).astype(NPBF),
        ))
    return TL, TH, per_core


_NC_CACHE = {}


def _cached(key, build, cfg):
    if key not in _NC_CACHE:
        _NC_CACHE[key] = build(cfg)
    return _NC_CACHE[key]


def _run(nc, in_maps, **kw):
    return run_bass_kernel_spmd(nc, in_maps, list(range(N_CORES)), **kw)


def kernel(static_emb, dyn0, dyn1, src_indices, trg_indices,
           w0, asrc0, atrg0, w1, asrc1, atrg1, _cfg=None, _runner=None):
    cfg = dict(_cfg_full() if _cfg is None else _cfg)
    N, NPC, W, NP, NPF = cfg["N"], cfg["NPC"], cfg["W"], cfg["NP"], cfg["NPF"]
    SPLIT = cfg["SPLIT"]
    run = _runner if _runner is not None else _run

    f32 = np.float32
    emb = np.concatenate([np.asarray(dyn0, f32), np.asarray(dyn1, f32),
                          np.asarray(static_emb, f32)], axis=1)  # [N, 128]
    embT = np.ascontiguousarray(emb.T)  # [128, N]

    w0 = np.asarray(w0, f32)
    w0e = np.zeros((128, 144), f32)
    w0e[:, :128] = w0.transpose(1, 0, 2).reshape(128, 128)
    w0e[:, 128:136] = np.einsum("hfd,hd->fh", w0, np.asarray(asrc0, f32)[:, :, 0])
    w0e[:, 136:144] = np.einsum("hfd,hd->fh", w0, np.asarray(atrg0, f32)[:, :, 0])

    w1 = np.asarray(w1, f32)
    w1e = np.zeros((128, 4), f32)
    w1e[:, 0:2] = w1[0]
    w1e[:, 2] = w1[0] @ np.asarray(asrc1, f32)[0, :, 0]
    w1e[:, 3] = w1[0] @ np.asarray(atrg1, f32)[0, :, 0]

    TL, TH, eprep = _prep_edges(src_indices, trg_indices, cfg)
    cfg["TL"], cfg["TH"] = TL, TH
    cfg["H"] = 8

    iota = np.broadcast_to(np.arange(128, dtype=f32), (128, 128)).astype(NPBF)
    iotac = np.arange(128, dtype=f32).reshape(128, 1)

    # ---- L1: sharded projection -----------------------------------------
    nc1 = _cached(("l1", NP), _build_l1, cfg)
    in1 = []
    for c in range(N_CORES):
        eo = np.zeros((128, NP), f32)
        eo[:, :NPC] = embT[:, c * NPC:(c + 1) * NPC]
        in1.append(dict(embT=eo, w0e=w0e))
    r1 = run(nc1, in1)

    hpS = np.concatenate([r1.results[c]["hpS"][:NPC] for c in range(N_CORES)])
    hpL = np.ascontiguousarray(hpS[:SPLIT])
    hpH = np.ascontiguousarray(hpS[SPLIT:])  # bf16 [_, 256] rows

    # ---- L2: layer-0 edge phase -----------------------------------------
    nc2 = _cached(("l2", NP, TL, TH), _build_l2, cfg)
    in2 = []
    for c in range(N_CORES):
        in2.append(dict(hpL=hpL, hpH=hpH, tS=r1.results[c]["tS"],
                        idxlo=eprep[c]["idxlo"], idxhi=eprep[c]["idxhi"],
                        tg=eprep[c]["tg"], tgr=eprep[c]["tgr"],
                        iota=iota, iotac=iotac))
    r2 = run(nc2, in2)

    h1 = np.zeros((NPF, 128), f32)
    for c in range(N_CORES):
        h1[c * NPC:(c + 1) * NPC] = r2.results[c]["h1"][:NPC]
    h1T = np.ascontiguousarray(h1.T)  # [128, NPF]

    # ---- L3: layer 1 + log_softmax --------------------------------------
    nc3 = _cached(("l3", NP, TL, TH), _build_l3, cfg)
    in3 = []
    for c in range(N_CORES):
        ho = np.zeros((128, NP), f32)
        ho[:, :NPC] = h1T[:, c * NPC:(c + 1) * NPC]
        in3.append(dict(h1T=h1T, h1To=ho, w1e=w1e,
                        idxlo=eprep[c]["idxlo"], idxhi=eprep[c]["idxhi"],
                        tg=eprep[c]["tg"], tgr=eprep[c]["tgr"],
                        iota=iota, iotac=iotac))
    r3 = run(nc3, in3)

    out = np.zeros((N, 2), f32)
    for c in range(N_CORES):
        o = r3.results[c]["outp"]  # [128, W, 2]; node = w*128 + p (local)
        loc = np.transpose(o, (1, 0, 2)).reshape(NP, 2)
        out[c * NPC:(c + 1) * NPC] = loc[:NPC]
    return out


# revision 8
# speedup vs baseline: 4.1236x; 1.1889x over previous
"""DenseSparseGAT (2-layer SpGAT, N=50000, E=800000) on 8 trn2 NeuronCores.

Strategy (graph/data parallel, per the sharding hint):
  - Nodes partitioned into 8 contiguous blocks of 6250; edges assigned to the
    core owning their destination (trg) node, sorted by trg within a core.
  - Per-edge source-node features fetched with SWDGE dma_gather from an
    HBM-resident projected-feature table (full table per core = replicated
    halo). Gathers are issued per 128-node window on rotating SWDGE queues
    (4 queues) so descriptor rings drain in parallel.
  - Destination-side attention terms are NOT gathered: within a window the
    trg values span 128 nodes, so t[trg] is expanded with a one-hot matmul
    (P_T[node, edge] built from a broadcast + is_equal, then PE matmul
    against the window's t rows).
  - segment_sum is a one-hot matmul into PSUM per window: lhsT = P[edge,
    node] one-hot (built with one is_equal per 128-edge tile), rhs =
    [msg | exp] packed bf16.
  - Softmax max-subtraction is skipped: alpha = exp(e)/sum(exp(e)) is
    shift-invariant and the scores are small enough that fp32 exp cannot
    overflow here.
  - Three SPMD launches: L1 projection (sharded), L2 layer-0 edge phase,
    L3 layer-1 (projection + edge phase + log_softmax). Host only does
    layout work (concat/transpose/sort/pad) between launches.
"""

import numpy as np

from concourse import bacc, bass, mybir
from concourse.tile import TileContext
from concourse.bass_utils import run_bass_kernel_spmd

DT = mybir.dt.float32
BF = mybir.dt.bfloat16
I16 = mybir.dt.int16
AF = mybir.ActivationFunctionType
ALU = mybir.AluOpType
NPBF = mybir.dt.np(BF)

N_CORES = 8
NQ = 4  # SWDGE queues


def _cfg_full():
    N = 50000
    NPC = N // N_CORES            # 6250 nodes per core
    W = (NPC + 127) // 128        # 49 windows
    NP = W * 128                  # 6272 padded nodes per core
    NPF = ((N + 127) // 128) * 128  # 50048 padded total nodes
    return dict(N=N, NPC=NPC, W=W, NP=NP, NPF=NPF, SPLIT=N // 2, H=8)


def _wrap_idx(a):
    """[num] ints -> dma_gather idx layout [128, num//16] int16 (replicated)."""
    num = a.shape[0]
    assert num % 16 == 0
    blk = np.ascontiguousarray(a.reshape(num // 16, 16).T.astype(np.int16))
    return np.tile(blk, (8, 1))


# --------------------------------------------------------------------------
# kernel builders
# --------------------------------------------------------------------------

def _build_l1(cfg):
    """Projection: hpS[n, :144] = [hp(128) | s(8) | t(8)], tS[n, :8] = t."""
    W, NP = cfg["W"], cfg["NP"]
    nc = bacc.Bacc(None, target_bir_lowering=False)
    embT = nc.dram_tensor("embT", [128, NP], DT, kind="ExternalInput")
    w0e = nc.dram_tensor("w0e", [128, 144], DT, kind="ExternalInput")
    hpS = nc.dram_tensor("hpS", [NP, 256], BF, kind="ExternalOutput")
    tS = nc.dram_tensor("tS", [NP, 64], DT, kind="ExternalOutput")

    B = 4  # chunks per DMA batch
    with TileContext(nc) as tc:
        with (
            tc.tile_pool(name="cs", bufs=1) as cs,
            tc.tile_pool(name="sb", bufs=3) as sb,
            tc.tile_pool(name="ps", bufs=2, space="PSUM") as psp,
        ):
            w0t = cs.tile([128, 144], DT)
            nc.sync.dma_start(w0t[:], w0e[:])
            for c0 in range(0, W, B):
                nb = min(B, W - c0)
                et = sb.tile([128, B * 128], DT, tag="et")
                nc.sync.dma_start(et[:, 0:nb * 128],
                                  embT[:, c0 * 128:(c0 + nb) * 128])
                ot = sb.tile([128, B, 16], DT, tag="ot")
                otb = sb.tile([128, B, 128], BF, tag="otb")
                for i in range(nb):
                    ps = psp.tile([128, 144], DT)
                    nc.tensor.matmul(ps[:], lhsT=et[:, i * 128:(i + 1) * 128],
                                     rhs=w0t[:], start=True, stop=True)
                    nc.vector.tensor_copy(ot[:, i, :], ps[:, 128:144])
                    nc.vector.tensor_copy(otb[:, i, :], ps[:, 0:128])
                r0, r1 = c0 * 128, (c0 + nb) * 128
                nc.sync.dma_start(
                    hpS[r0:r1, 0:128].rearrange("(c p) d -> p c d", p=128),
                    otb[:, 0:nb, :])
                nc.scalar.dma_start(
                    hpS[r0:r1, 128:144].bitcast(DT)
                    .rearrange("(c p) d -> p c d", p=128),
                    ot[:, 0:nb, 0:8])
                nc.scalar.dma_start(
                    tS[r0:r1, 0:8].rearrange("(c p) d -> p c d", p=128),
                    ot[:, 0:nb, 8:16])
    nc.compile()
    return nc


def _edge_phase(nc, tc, cfg, src_lo, src_hi, tdram, idxlo, idxhi, tg_d, tgr_d,
                iota_d, iotac_d, gdt, gcols, s_view, ncol, post_fn):
    """Shared edge phase for one GAT layer.

    src_lo/src_hi: DRAM tables gathered by (split) src index; fp32 rows of
      gcols (msg features at [0:ncol], s at [scol0:scol0+H]).
    tdram: fp32 [NP, 64] table, t at cols [0:H], read per window (no gather).
    """
    W, H = cfg["W"], cfg["H"]
    W_RUN = cfg.get("W_RUN", W)
    TL, TH = cfg["TL"], cfg["TH"]
    TW = TL + TH
    EW = TW * 128  # edge slots per window
    qctr = [0]

    def gather(dst, srcd, idxs, col0, ntiles):
        # two balanced chunks (<= 640 idx each: fits the SWDGE ring, and
        # heavy calls spread across all 4 queues)
        half = (ntiles + 1) // 2
        for j in range(0, ntiles, half):
            k = min(half, ntiles - j)
            nc.gpsimd.dma_gather(
                dst[:, j:j + k, :], srcd[:],
                idxs[:, col0 + j * 8:col0 + (j + k) * 8],
                k * 128, k * 128, gcols, queue_num=qctr[0] % NQ)
            qctr[0] += 1

    with (
        tc.tile_pool(name="ecs", bufs=1) as cs,
        tc.tile_pool(name="eg", bufs=4) as pg,
        tc.tile_pool(name="esm", bufs=2) as psm,
        tc.tile_pool(name="epp", bufs=2) as pp,
        tc.tile_pool(name="eps", bufs=2, space="PSUM") as psp,
    ):
        ilo = cs.tile([128, W * TL * 8], I16)
        nc.sync.dma_start(ilo[:], idxlo[:])
        ihi = cs.tile([128, W * TH * 8], I16)
        nc.sync.dma_start(ihi[:], idxhi[:])
        tg = cs.tile([128, W * TW], DT)
        nc.sync.dma_start(tg[:], tg_d[:])
        io = cs.tile([128, 128], BF)
        nc.sync.dma_start(io[:], iota_d[:])
        ioc = cs.tile([128, 1], DT)
        nc.sync.dma_start(ioc[:], iotac_d[:])
        ones = cs.tile([1, 128], BF)
        nc.gpsimd.memset(ones[:], 1.0)

        for w in range(W_RUN):
            glo = pg.tile([128, TL, gcols], gdt, tag="glo")
            gather(glo, src_lo, ilo, w * TL * 8, TL)
            ghi = pg.tile([128, TH, gcols], gdt, tag="ghi")
            gather(ghi, src_hi, ihi, w * TH * 8, TH)

            # window t rows + trgl broadcast row
            twf = psm.tile([128, H], DT, tag="twf")
            nc.scalar.dma_start(twf[:], tdram[w * 128:(w + 1) * 128, 0:H])
            twb = psm.tile([128, H], BF, tag="twb")
            nc.vector.tensor_copy(twb[:], twf[:])
            tgr = psm.tile([1, EW], BF, tag="tgr")
            nc.scalar.dma_start(tgr[:], tgr_d[:, w * EW:(w + 1) * EW])

            # P_T[node, edge] one-hot: broadcast trgl row, compare to iota col
            PT = psm.tile([128, EW], BF, tag="PT")
            for o in range(0, EW, 512):
                S = min(512, EW - o)
                psb = psp.tile([128, 512], DT, tag="psb")
                nc.tensor.matmul(psb[:, 0:S], lhsT=ones[:], rhs=tgr[:, o:o + S],
                                 start=True, stop=True)
                nc.vector.tensor_scalar(
                    out=PT[:, o:o + S], in0=psb[:, 0:S], scalar1=ioc[:],
                    scalar2=None, op0=ALU.is_equal)

            # t_edge[edge, h] = P_T^T @ t_win, per 128-edge tile into one bank
            psT = psp.tile([128, TW * H], DT, tag="psT")
            for t in range(TW):
                nc.tensor.matmul(psT[:, t * H:(t + 1) * H],
                                 lhsT=PT[:, t * 128:(t + 1) * 128],
                                 rhs=twb[:], start=True, stop=True)

            # e = leaky_relu(s_src + t_trg, 0.2); rhs = [msg | exp] bf16
            e = psm.tile([128, TW, H], DT, tag="e")
            nc.vector.tensor_tensor(
                out=e[:, 0:TL, :], in0=s_view(glo),
                in1=psT[:, 0:TL * H].rearrange("p (t h) -> p t h", h=H),
                op=ALU.add)
            nc.vector.tensor_tensor(
                out=e[:, TL:TW, :], in0=s_view(ghi),
                in1=psT[:, TL * H:TW * H].rearrange("p (t h) -> p t h", h=H),
                op=ALU.add)
            el = psm.tile([128, TW, H], DT, tag="el")
            nc.vector.scalar_tensor_tensor(
                out=el[:], in0=e[:], scalar=0.2, in1=e[:],
                op0=ALU.mult, op1=ALU.max)
            rhs = psm.tile([128, TW, ncol + H], BF, tag="rhs")
            nc.scalar.activation(rhs[:, :, ncol:ncol + H], el[:], AF.Exp)

            # msg = hp_src * exp (per head block), bf16
            dsub = ncol // H
            nc.vector.tensor_tensor(
                out=rhs[:, 0:TL, 0:ncol].rearrange("p t (h d) -> p t h d", d=dsub),
                in0=glo[:, :, 0:ncol].rearrange("p t (h d) -> p t h d", d=dsub),
                in1=rhs[:, 0:TL, ncol:ncol + H, None]
                    .to_broadcast([128, TL, H, dsub]),
                op=ALU.mult)
            nc.vector.tensor_tensor(
                out=rhs[:, TL:TW, 0:ncol].rearrange("p t (h d) -> p t h d", d=dsub),
                in0=ghi[:, :, 0:ncol].rearrange("p t (h d) -> p t h d", d=dsub),
                in1=rhs[:, TL:TW, ncol:ncol + H, None]
                    .to_broadcast([128, TH, H, dsub]),
                op=ALU.mult)

            # segment sum: accumulate [num | den] over the window's tiles
            Pall = pp.tile([128, TW, 128], BF, tag="Pall")
            nc.vector.tensor_tensor(
                out=Pall[:],
                in0=io[:, None, :].to_broadcast([128, TW, 128]),
                in1=tg[:, w * TW:(w + 1) * TW, None].to_broadcast([128, TW, 128]),
                op=ALU.is_equal)
            psA = psp.tile([128, ncol + H], DT, tag="psA")
            for t in range(TW):
                nc.tensor.matmul(psA[:], lhsT=Pall[:, t, :], rhs=rhs[:, t, :],
                                 start=(t == 0), stop=(t == TW - 1))

            post_fn(w, psA)


def _build_l2(cfg):
    """Layer-0 edge phase: gathers from hpS (lo/hi split), t expanded; out h1."""
    W, NP, H = cfg["W"], cfg["NP"], cfg["H"]
    SPLIT, N = cfg["SPLIT"], cfg["N"]
    TL, TH = cfg["TL"], cfg["TH"]
    nc = bacc.Bacc(None, target_bir_lowering=False, num_swdge_queues=NQ)
    hpL = nc.dram_tensor("hpL", [SPLIT, 256], BF, kind="ExternalInput")
    hpH = nc.dram_tensor("hpH", [N - SPLIT, 256], BF, kind="ExternalInput")
    tS = nc.dram_tensor("tS", [NP, 64], DT, kind="ExternalInput")
    idxlo = nc.dram_tensor("idxlo", [128, W * TL * 8], I16, kind="ExternalInput")
    idxhi = nc.dram_tensor("idxhi", [128, W * TH * 8], I16, kind="ExternalInput")
    tg_d = nc.dram_tensor("tg", [128, W * (TL + TH)], DT, kind="ExternalInput")
    tgr_d = nc.dram_tensor("tgr", [1, W * (TL + TH) * 128], BF,
                           kind="ExternalInput")
    iota_d = nc.dram_tensor("iota", [128, 128], BF, kind="ExternalInput")
    iotac_d = nc.dram_tensor("iotac", [128, 1], DT, kind="ExternalInput")
    h1 = nc.dram_tensor("h1", [NP, 128], DT, kind="ExternalOutput")

    with TileContext(nc) as tc:
        with tc.tile_pool(name="post", bufs=3) as pq:
            def post(w, psA):
                # h0 = num / (den + 1e-16); h1 = elu(h0)
                dn = pq.tile([128, H], DT, tag="dn")
                nc.vector.tensor_scalar(out=dn[:], in0=psA[:, 128:128 + H],
                                        scalar1=1e-16, scalar2=None, op0=ALU.add)
                rc = pq.tile([128, H], DT, tag="rc")
                nc.vector.reciprocal(rc[:], dn[:])
                h0 = pq.tile([128, 128], DT, tag="h0")
                nc.vector.tensor_tensor(
                    out=h0[:].rearrange("p (h d) -> p h d", d=16),
                    in0=psA[:, 0:128].rearrange("p (h d) -> p h d", d=16),
                    in1=rc[:, :, None].to_broadcast([128, H, 16]),
                    op=ALU.mult)
                hm = pq.tile([128, 128], DT, tag="hm")
                nc.vector.tensor_scalar(out=hm[:], in0=h0[:], scalar1=0.0,
                                        scalar2=None, op0=ALU.min)
                he = pq.tile([128, 128], DT, tag="he")
                nc.scalar.activation(he[:], hm[:], AF.Exp)
                ho = pq.tile([128, 128], DT, tag="ho")
                nc.vector.scalar_tensor_tensor(
                    out=ho[:], in0=h0[:], scalar=0.0, in1=he[:],
                    op0=ALU.max, op1=ALU.add)
                nc.vector.tensor_scalar(out=ho[:], in0=ho[:], scalar1=1.0,
                                        scalar2=None, op0=ALU.subtract)
                nc.sync.dma_start(h1[w * 128:(w + 1) * 128, :], ho[:])

            _edge_phase(nc, tc, cfg, hpL, hpH, tS, idxlo, idxhi, tg_d, tgr_d,
                        iota_d, iotac_d, gdt=BF, gcols=256,
                        s_view=lambda g: g[:, :, 128:144].bitcast(DT),
                        ncol=128, post_fn=post)
    nc.compile()
    return nc


def _build_l3(cfg):
    """Layer 1: full projection of h1 -> hp1S, edge phase, log_softmax."""
    W, NP, NPF = cfg["W"], cfg["NP"], cfg["NPF"]
    SPLIT = cfg["SPLIT"]
    TL, TH = cfg["TL"], cfg["TH"]
    CH = NPF // 128
    B = 4
    nc = bacc.Bacc(None, target_bir_lowering=False, num_swdge_queues=NQ)
    h1T = nc.dram_tensor("h1T", [128, NPF], DT, kind="ExternalInput")
    h1To = nc.dram_tensor("h1To", [128, NP], DT, kind="ExternalInput")
    w1e = nc.dram_tensor("w1e", [128, 4], DT, kind="ExternalInput")
    idxlo = nc.dram_tensor("idxlo", [128, W * TL * 8], I16, kind="ExternalInput")
    idxhi = nc.dram_tensor("idxhi", [128, W * TH * 8], I16, kind="ExternalInput")
    tg_d = nc.dram_tensor("tg", [128, W * (TL + TH)], DT, kind="ExternalInput")
    tgr_d = nc.dram_tensor("tgr", [1, W * (TL + TH) * 128], BF,
                           kind="ExternalInput")
    iota_d = nc.dram_tensor("iota", [128, 128], BF, kind="ExternalInput")
    iotac_d = nc.dram_tensor("iotac", [128, 1], DT, kind="ExternalInput")
    outp = nc.dram_tensor("outp", [128, W, 2], DT, kind="ExternalOutput")

    hp1L = nc.dram_tensor("hp1L", [SPLIT, 64], DT)
    hp1H = nc.dram_tensor("hp1H", [NPF - SPLIT, 64], DT)
    t1S = nc.dram_tensor("t1S", [NP, 64], DT)

    with TileContext(nc) as tc:
        # (a) full projection hp1S rows = [hp1(2) | s1(1) | t1(1)]
        with (
            tc.tile_pool(name="pcs", bufs=1) as cs,
            tc.tile_pool(name="psb", bufs=3) as sb,
            tc.tile_pool(name="pps", bufs=2, space="PSUM") as psp,
        ):
            w1t = cs.tile([128, 4], DT)
            nc.sync.dma_start(w1t[:], w1e[:])
            for c0 in range(0, CH, B):
                nb = min(B, CH - c0)
                ht = sb.tile([128, B * 128], DT, tag="ht")
                nc.sync.dma_start(ht[:, 0:nb * 128],
                                  h1T[:, c0 * 128:(c0 + nb) * 128])
                ot = sb.tile([128, B, 4], DT, tag="ot")
                ps = psp.tile([128, B * 4], DT, tag="psa")
                for i in range(nb):
                    nc.tensor.matmul(ps[:, i * 4:(i + 1) * 4],
                                     lhsT=ht[:, i * 128:(i + 1) * 128],
                                     rhs=w1t[:], start=True, stop=True)
                nc.vector.tensor_copy(
                    ot[:, 0:nb, :],
                    ps[:, 0:nb * 4].rearrange("p (c d) -> p c d", d=4))
                r0, r1 = c0 * 128, (c0 + nb) * 128
                eng = nc.sync if (c0 // B) % 2 == 0 else nc.scalar
                if r1 <= SPLIT:
                    eng.dma_start(
                        hp1L[r0:r1, 0:4].rearrange("(c p) d -> p c d", p=128),
                        ot[:, 0:nb, :])
                elif r0 >= SPLIT:
                    eng.dma_start(
                        hp1H[r0 - SPLIT:r1 - SPLIT, 0:4]
                        .rearrange("(c p) d -> p c d", p=128),
                        ot[:, 0:nb, :])
                else:
                    for i in range(nb):
                        rr0 = r0 + i * 128
                        if rr0 + 128 <= SPLIT:
                            eng.dma_start(hp1L[rr0:rr0 + 128, 0:4], ot[:, i, :])
                        elif rr0 >= SPLIT:
                            eng.dma_start(
                                hp1H[rr0 - SPLIT:rr0 - SPLIT + 128, 0:4],
                                ot[:, i, :])
                        else:
                            k = SPLIT - rr0
                            eng.dma_start(hp1L[rr0:SPLIT, 0:4], ot[0:k, i, :])
                            eng.dma_start(hp1H[0:rr0 + 128 - SPLIT, 0:4],
                                          ot[k:128, i, :])

            # (b) t1 for own nodes -> t1S col 0:1
            for c0 in range(0, W, B):
                nb = min(B, W - c0)
                ht = sb.tile([128, B * 128], DT, tag="ht")
                nc.sync.dma_start(ht[:, 0:nb * 128],
                                  h1To[:, c0 * 128:(c0 + nb) * 128])
                ot = sb.tile([128, B, 4], DT, tag="ot")
                ps = psp.tile([128, B * 4], DT, tag="psa")
                for i in range(nb):
                    nc.tensor.matmul(ps[:, i * 4:(i + 1) * 4],
                                     lhsT=ht[:, i * 128:(i + 1) * 128],
                                     rhs=w1t[:], start=True, stop=True)
                nc.vector.tensor_copy(
                    ot[:, 0:nb, :],
                    ps[:, 0:nb * 4].rearrange("p (c d) -> p c d", d=4))
                r0, r1 = c0 * 128, (c0 + nb) * 128
                nc.scalar.dma_start(
                    t1S[r0:r1, 0:1].rearrange("(c p) d -> p c d", p=128),
                    ot[:, 0:nb, 3:4])

        # (c) edge phase; accumulate per-window logits
        with tc.tile_pool(name="post", bufs=1) as pq:
            acc = pq.tile([128, W, 3], DT)

            def post(w, psA):
                nc.vector.tensor_copy(acc[:, w, :], psA[:, 0:3])

            cfg3 = dict(cfg)
            cfg3["H"] = 1
            _edge_phase(nc, tc, cfg3, hp1L, hp1H, t1S, idxlo, idxhi, tg_d,
                        tgr_d, iota_d, iotac_d, gdt=DT, gcols=64,
                        s_view=lambda g: g[:, :, 2:3], ncol=2, post_fn=post)

            # (d) batched normalize + log_softmax
            dn = pq.tile([128, W], DT)
            nc.vector.tensor_scalar(out=dn[:], in0=acc[:, :, 2], scalar1=1e-16,
                                    scalar2=None, op0=ALU.add)
            rc = pq.tile([128, W], DT)
            nc.vector.reciprocal(rc[:], dn[:])
            lg = pq.tile([128, W, 2], DT)
            nc.vector.tensor_tensor(out=lg[:], in0=acc[:, :, 0:2],
                                    in1=rc[:, :, None].to_broadcast([128, W, 2]),
                                    op=ALU.mult)
            mx = pq.tile([128, W], DT)
            nc.vector.tensor_reduce(out=mx[:], in_=lg[:],
                                    axis=mybir.AxisListType.X, op=ALU.max)
            dd = pq.tile([128, W, 2], DT)
            nc.vector.tensor_tensor(out=dd[:], in0=lg[:],
                                    in1=mx[:, :, None].to_broadcast([128, W, 2]),
                                    op=ALU.subtract)
            e2 = pq.tile([128, W, 2], DT)
            nc.scalar.activation(e2[:], dd[:], AF.Exp)
            se = pq.tile([128, W], DT)
            nc.vector.tensor_reduce(out=se[:], in_=e2[:],
                                    axis=mybir.AxisListType.X, op=ALU.add)
            ls = pq.tile([128, W], DT)
            nc.scalar.activation(ls[:], se[:], AF.Ln)
            ov = pq.tile([128, W, 2], DT)
            nc.vector.tensor_tensor(out=ov[:], in0=dd[:],
                                    in1=ls[:, :, None].to_broadcast([128, W, 2]),
                                    op=ALU.subtract)
            nc.sync.dma_start(outp[:], ov[:])
    nc.compile()
    return nc


# --------------------------------------------------------------------------
# host-side prep (index/layout only)
# --------------------------------------------------------------------------

def _prep_edges(src, trg, cfg):
    """Partition+sort edges by destination; build per-core gather index and
    one-hot-builder arrays. Returns (TL, TH, per_core list of dicts)."""
    N, NPC, W, SPLIT = cfg["N"], cfg["NPC"], cfg["W"], cfg["SPLIT"]
    src = np.asarray(src).astype(np.int64)
    trg = np.asarray(trg).astype(np.int64)

    cores = []
    max_lo = 1
    max_hi = 1
    for c in range(N_CORES):
        m = (trg >= c * NPC) & (trg < (c + 1) * NPC)
        es, et = src[m], trg[m] - c * NPC
        o = np.argsort(et, kind="stable")
        es, et = es[o], et[o]
        bounds = np.searchsorted(et, np.arange(W + 1) * 128)
        wins = []
        for w in range(W):
            ws, wt = es[bounds[w]:bounds[w + 1]], et[bounds[w]:bounds[w + 1]]
            lo = ws < SPLIT
            wins.append((ws[lo], wt[lo], ws[~lo] - SPLIT, wt[~lo]))
            max_lo = max(max_lo, int(lo.sum()))
            max_hi = max(max_hi, int((~lo).sum()))
        cores.append(wins)

    TL = (max_lo + 127) // 128
    TH = (max_hi + 127) // 128
    TW = TL + TH

    per_core = []
    for c in range(N_CORES):
        idxlo = np.zeros((W, TL * 128), np.int64)
        idxhi = np.zeros((W, TH * 128), np.int64)
        trgl = np.full((W, TW * 128), -1.0, np.float32)
        for w, (slo, tlo, shi, thi) in enumerate(cores[c]):
            nlo, nhi = len(slo), len(shi)
            idxlo[w, :nlo] = slo
            trgl[w, :nlo] = tlo - 128 * w
            idxhi[w, :nhi] = shi
            trgl[w, TL * 128:TL * 128 + nhi] = thi - 128 * w
        per_core.append(dict(
            idxlo=np.concatenate([_wrap_idx(idxlo[w]) for w in range(W)], axis=1),
            idxhi=np.concatenate([_wrap_idx(idxhi[w]) for w in range(W)], axis=1),
            tg=np.ascontiguousarray(
                np.stack([trgl[w].reshape(TW, 128).T for w in range(W)], axis=1)
                .reshape(128, W * TW)),
            tgr=np.ascontiguousarray(trgl.reshape(1, W * TW * 128)).astype(NPBF),
        ))
    return TL, TH, per_core


_NC_CACHE = {}


def _cached(key, build, cfg):
    if key not in _NC_CACHE:
        _NC_CACHE[key] = build(cfg)
    return _NC_CACHE[key]


def _run(nc, in_maps, **kw):
    return run_bass_kernel_spmd(nc, in_maps, list(range(N_CORES)), **kw)


def kernel(static_emb, dyn0, dyn1, src_indices, trg_indices,
           w0, asrc0, atrg0, w1, asrc1, atrg1, _cfg=None, _runner=None):
    cfg = dict(_cfg_full() if _cfg is None else _cfg)
    N, NPC, W, NP, NPF = cfg["N"], cfg["NPC"], cfg["W"], cfg["NP"], cfg["NPF"]
    SPLIT = cfg["SPLIT"]
    run = _runner if _runner is not None else _run

    f32 = np.float32
    emb = np.concatenate([np.asarray(dyn0, f32), np.asarray(dyn1, f32),
                          np.asarray(static_emb, f32)], axis=1)  # [N, 128]
    embT = np.ascontiguousarray(emb.T)  # [128, N]

    w0 = np.asarray(w0, f32)
    w0e = np.zeros((128, 144), f32)
    w0e[:, :128] = w0.transpose(1, 0, 2).reshape(128, 128)
    w0e[:, 128:136] = np.einsum("hfd,hd->fh", w0, np.asarray(asrc0, f32)[:, :, 0])
    w0e[:, 136:144] = np.einsum("hfd,hd->fh", w0, np.asarray(atrg0, f32)[:, :, 0])

    w1 = np.asarray(w1, f32)
    w1e = np.zeros((128, 4), f32)
    w1e[:, 0:2] = w1[0]
    w1e[:, 2] = w1[0] @ np.asarray(asrc1, f32)[0, :, 0]
    w1e[:, 3] = w1[0] @ np.asarray(atrg1, f32)[0, :, 0]

    TL, TH, eprep = _prep_edges(src_indices, trg_indices, cfg)
    cfg["TL"], cfg["TH"] = TL, TH
    cfg["H"] = 8

    iota = np.broadcast_to(np.arange(128, dtype=f32), (128, 128)).astype(NPBF)
    iotac = np.arange(128, dtype=f32).reshape(128, 1)

    # ---- L1: sharded projection -----------------------------------------
    nc1 = _cached(("l1", NP), _build_l1, cfg)
    in1 = []
    for c in range(N_CORES):
        eo = np.zeros((128, NP), f32)
        eo[:, :NPC] = embT[:, c * NPC:(c + 1) * NPC]
        in1.append(dict(embT=eo, w0e=w0e))
    r1 = run(nc1, in1)

    hpS = np.concatenate([r1.results[c]["hpS"][:NPC] for c in range(N_CORES)])
    hpL = np.ascontiguousarray(hpS[:SPLIT])
    hpH = np.ascontiguousarray(hpS[SPLIT:])  # bf16 [_, 256] rows

    # ---- L2: layer-0 edge phase -----------------------------------------
    nc2 = _cached(("l2", NP, TL, TH), _build_l2, cfg)
    in2 = []
    for c in range(N_CORES):
        in2.append(dict(hpL=hpL, hpH=hpH, tS=r1.results[c]["tS"],
                        idxlo=eprep[c]["idxlo"], idxhi=eprep[c]["idxhi"],
                        tg=eprep[c]["tg"], tgr=eprep[c]["tgr"],
                        iota=iota, iotac=iotac))
    r2 = run(nc2, in2)

    h1 = np.zeros((NPF, 128), f32)
    for c in range(N_CORES):
        h1[c * NPC:(c + 1) * NPC] = r2.results[c]["h1"][:NPC]
    h1T = np.ascontiguousarray(h1.T)  # [128, NPF]

    # ---- L3: layer 1 + log_softmax --------------------------------------
    nc3 = _cached(("l3", NP, TL, TH), _build_l3, cfg)
    in3 = []
    for c in range(N_CORES):
        ho = np.zeros((128, NP), f32)
        ho[:, :NPC] = h1T[:, c * NPC:(c + 1) * NPC]
        in3.append(dict(h1T=h1T, h1To=ho, w1e=w1e,
                        idxlo=eprep[c]["idxlo"], idxhi=eprep[c]["idxhi"],
                        tg=eprep[c]["tg"], tgr=eprep[c]["tgr"],
                        iota=iota, iotac=iotac))
    r3 = run(nc3, in3)

    out = np.zeros((N, 2), f32)
    for c in range(N_CORES):
        o = r3.results[c]["outp"]  # [128, W, 2]; node = w*128 + p (local)
        loc = np.transpose(o, (1, 0, 2)).reshape(NP, 2)
        out[c * NPC:(c + 1) * NPC] = loc[:NPC]
    return out
